# revision 18
# baseline (speedup 1.0000x reference)
"""Cosformer attention Bass kernel for 8 trn2 NeuronCores — fp8 edition.

Sharding: core c handles batch c//2, sequence half c%2 (1024 tokens).
Per-head linear-attention state (kv, ksum) is AllReduce'd (bf16) between
the two cores sharing a batch.

The 4 big projection matmuls run in fp8e4m3 with DoubleRow perf mode
(K=256 per instruction, ~2x bf16 rate). Weights are scaled x64 on host to
escape the fp8 subnormal zone; the scales cancel through the cosformer
z-normalization or are folded into epilogues.

LN1 folding: rstd cancels through z-normalization (relu commutes with
positive per-token scale), so the Q projection consumes centered x only:
x_c = x - mu, built feature-major from a partition-broadcast of mu. This
removes the qn transposes entirely; qn stays SBUF-resident for the
residual.

The final residual (+ out) is applied as an epilogue add (NOT folded into
Wo: the identity diagonal would quantize at ~6% in fp8).
"""

import sys

for _p in ('/opt/trn_rl_repo',):
    if _p not in sys.path:
        sys.path.insert(0, _p)

import importlib.util as _ilu
import os

os.environ.setdefault('NEURON_RT_RESET_CORES', '1')

# The image's antenv lacks axon_hooks (needed for trace=True); register ours.
if 'antenv.axon_hooks' not in sys.modules:
    _hp = '/opt/trn_rl_repo/antenv/axon_hooks.py'
    if os.path.exists(_hp):
        _spec = _ilu.spec_from_file_location('antenv.axon_hooks', _hp)
        _mod = _ilu.module_from_spec(_spec)
        _spec.loader.exec_module(_mod)
        sys.modules['antenv.axon_hooks'] = _mod
    else:
        import types as _types

        _mod = _types.ModuleType('antenv.axon_hooks')
        _mod._hook = None
        _mod.set_axon_ntff_profile_hook = lambda h: setattr(_mod, '_hook', h)
        _mod.get_axon_ntff_profile_hook = lambda: _mod._hook
        sys.modules['antenv.axon_hooks'] = _mod


def _register_ntff_hook():
    """If boot didn't register the NTFF profile hook (image antenv lacks
    axon_hooks), drive NRT profiling via ctypes into libaxon_pjrt.so."""
    import contextlib
    import ctypes

    mod = sys.modules['antenv.axon_hooks']
    if mod.get_axon_ntff_profile_hook() is not None:
        return
    so_path = '/opt/axon/libaxon_pjrt.so'
    if not os.path.exists(so_path):
        return
    try:
        lib = ctypes.CDLL(so_path)
        if not hasattr(lib, 'axon_start_nrt_profile'):
            return
        lib.axon_start_nrt_profile.argtypes = [
            ctypes.POINTER(ctypes.c_int64), ctypes.c_size_t]
        lib.axon_start_nrt_profile.restype = ctypes.c_int64
        lib.axon_stop_nrt_profile.argtypes = [ctypes.c_char_p]
        lib.axon_stop_nrt_profile.restype = ctypes.c_int64
    except OSError:
        return

    @contextlib.contextmanager
    def _hook(output_dir, device_ids):
        import jax
        jax.devices()
        if device_ids:
            ids = (ctypes.c_int64 * len(device_ids))(*device_ids)
            rc = lib.axon_start_nrt_profile(ids, len(device_ids))
        else:
            rc = lib.axon_start_nrt_profile(None, 0)
        if rc != 0:
            raise RuntimeError(f'axon_start_nrt_profile rc={rc}')
        try:
            yield
        finally:
            n = lib.axon_stop_nrt_profile(str(output_dir).encode())
            if n < 0:
                raise RuntimeError(f'axon_stop_nrt_profile rc={n}')

    mod.set_axon_ntff_profile_hook(_hook)


_register_ntff_hook()

import numpy as np
import ml_dtypes

import concourse.bass as bass
import concourse.tile as tile
from concourse import bacc, mybir
from concourse.alu_op_type import AluOpType
from concourse.bass_utils import run_bass_kernel_spmd

BF16 = ml_dtypes.bfloat16
E4M3 = ml_dtypes.float8_e4m3
FP32 = mybir.dt.float32
BF = mybir.dt.bfloat16
F8 = mybir.dt.float8e4
AF = mybir.ActivationFunctionType
DR = mybir.MatmulPerfMode.DoubleRow

L, N, E, H, D = 2048, 4, 1024, 16, 64
T = 1024            # tokens per core
NT = T // 128       # 8 token tiles
NK2 = 4             # DoubleRow contraction steps (256 features each)
NJ = E // 128       # 8 output-feature tiles
NCORES = 8
EPS_LN = 1e-5
EPS_ATTN = 1e-6
WS = 64.0           # host weight scale
IWS = 1.0 / WS

_BUILD_CACHE = {}


def _build_program(flags):
    """flags: (has_g1b1, has_qb, has_kb, has_vb, has_g2, has_b2o)."""
    has_g1b1, has_qb, has_kb, has_vb, has_g2, has_b2o = flags
    general_q = has_g1b1 or has_qb   # r no longer cancels for the Q path

    nc = bacc.Bacc("TRN2", target_bir_lowering=False, debug=False,
                   num_devices=NCORES)

    # ---- DRAM I/O ----
    d_x_tm = nc.dram_tensor('x_tm', [T, E], BF, kind='ExternalInput')
    d_x_fm8 = nc.dram_tensor('x_fm8', [128, NK2, 2, T], F8, kind='ExternalInput')
    d_wq = nc.dram_tensor('wq8', [128, NK2, 2, E], F8, kind='ExternalInput')
    d_wk = nc.dram_tensor('wk8', [128, NK2, 2, E], F8, kind='ExternalInput')
    d_wv = nc.dram_tensor('wv8', [128, NK2, 2, E], F8, kind='ExternalInput')
    d_wo = nc.dram_tensor('wo8', [128, NK2, 2, E], F8, kind='ExternalInput')
    d_sb = nc.dram_tensor('s_bcast', [128, T], BF, kind='ExternalInput')
    d_cb = nc.dram_tensor('c_bcast', [128, T], BF, kind='ExternalInput')
    d_scol = nc.dram_tensor('s_cols', [128, NT], FP32, kind='ExternalInput')
    d_ccol = nc.dram_tensor('c_cols', [128, NT], FP32, kind='ExternalInput')
    d_g1b = nc.dram_tensor('g1_b', [128, E], FP32, kind='ExternalInput') if has_g1b1 else None
    d_b1b = nc.dram_tensor('b1_b', [128, E], FP32, kind='ExternalInput') if has_g1b1 else None
    d_qcj = nc.dram_tensor('q_cj', [128, NJ], FP32, kind='ExternalInput') if general_q else None
    d_kbb = nc.dram_tensor('kb_b', [128, E], FP32, kind='ExternalInput') if has_kb else None
    d_vbb = nc.dram_tensor('vb_b', [128, E], FP32, kind='ExternalInput') if has_vb else None
    d_g2b = nc.dram_tensor('g2_b', [128, E], FP32, kind='ExternalInput') if has_g2 else None
    d_b2ob = nc.dram_tensor('b2o_b', [128, E], FP32, kind='ExternalInput') if has_b2o else None
    d_out = nc.dram_tensor('out', [T, E], BF, kind='ExternalOutput')

    RG = [[0, 1], [2, 3], [4, 5], [6, 7]]

    with tile.TileContext(nc) as tc:
        with (
            tc.tile_pool(name='persist', bufs=1) as pp,
            tc.tile_pool(name='wpool', bufs=2) as wp,
            tc.tile_pool(name='dram', bufs=1, space='DRAM') as dp,
        ):
            # ---- persistent tiles ----
            sbt = pp.tile([128, T], BF, tag='sbt')          # s/WS bcast
            cbt = pp.tile([128, T], BF, tag='cbt')
            scol = pp.tile([128, NT], FP32, tag='scol')     # s/8 cols
            ccol = pp.tile([128, NT], FP32, tag='ccol')
            eps1 = pp.tile([128, 1], FP32, tag='eps1')
            qn = pp.tile([128, NT, E], BF, tag='qn')        # LN1 out, token-major
            xh = pp.tile([128, NT, E], BF, tag='xh')        # LN2 out, token-major
            xc8 = pp.tile([128, NK2, 2, T], F8, tag='xc8')  # centered x, fm
            ksc = pp.tile([128, NT, H, 128], F8, tag='ksc') # 8*k*[s|c], tok-major
            v_aug = pp.tile([128, NT, H, 65], F8, tag='vaug')
            qq = pp.tile([128, H, T], BF, tag='qq')         # q~*[s|c] per head, fm
            kvb = pp.tile([128, H * 65], BF, tag='kvb')     # reduced kv (true scale)
            kvp = pp.tile([128, H * 65], BF, tag='kvp')
            xhT = pp.tile([128, NJ, T], BF, tag='xhT')      # transposed xh
            xhT8 = pp.tile([128, NK2, 2, T], F8, tag='xhT8')
            negmu_cols = pp.tile([128, NT], BF, tag='nmu')  # -mu per token
            mu_row = pp.tile([1, T], BF, tag='murow')
            mu_bc = pp.tile([128, T], BF, tag='mubc')
            rst_cols = pp.tile([128, NT], FP32, tag='rst') if general_q else None
            r_row = pp.tile([1, T], FP32, tag='rrow') if general_q else None
            r_bc = pp.tile([128, T], FP32, tag='rbc') if general_q else None

            g1b = b1b = qcj = kbb = vbb = g2b = b2ob = None
            if has_g1b1:
                g1b = pp.tile([128, E], FP32, tag='g1b')
                b1b = pp.tile([128, E], FP32, tag='b1b')
                nc.gpsimd.dma_start(out=g1b, in_=d_g1b[:])
                nc.gpsimd.dma_start(out=b1b, in_=d_b1b[:])
            if general_q:
                qcj = pp.tile([128, NJ], FP32, tag='qcj')
                nc.gpsimd.dma_start(out=qcj, in_=d_qcj[:])
            if has_kb:
                kbb = pp.tile([128, E], FP32, tag='kbb')
                nc.gpsimd.dma_start(out=kbb, in_=d_kbb[:])
            if has_vb:
                vbb = pp.tile([128, E], FP32, tag='vbb')
                nc.gpsimd.dma_start(out=vbb, in_=d_vbb[:])
            if has_g2:
                g2b = pp.tile([128, E], FP32, tag='g2b')
                nc.gpsimd.dma_start(out=g2b, in_=d_g2b[:])
            if has_b2o:
                b2ob = pp.tile([128, E], FP32, tag='b2ob')
                nc.gpsimd.dma_start(out=b2ob, in_=d_b2ob[:])

            # DRAM scratch
            xh_dram = dp.tile([T, E], BF)
            kv_cc_in = dp.tile([128, H * 65], BF)
            kv_cc_out = dp.tile([128, H * 65], BF)

            # ---- front-loaded DMAs ----
            # scalar queue: x_tm tiles (LN1 needs them first) interleaved
            # with Wk chunks (PE needs chunk 0 at ~1.5us)
            xfm = pp.tile([128, NK2, 2, T], F8, tag='xfm')
            wk_t = wp.tile([128, NK2, 2, E], F8, tag='W')
            with tc.tile_pool(name='xtiles', bufs=8) as xp:
                xts = []
                for i in range(NT):
                    xt = xp.tile([128, E], BF, tag='xt', name=f'xt{i}')
                    xts.append(xt)
                nc.scalar.dma_start(out=xts[0], in_=d_x_tm[0:128, :])
                nc.scalar.dma_start(out=wk_t[:, 0], in_=d_wk[:, 0])
                nc.scalar.dma_start(out=xts[1], in_=d_x_tm[128:256, :])
                nc.scalar.dma_start(out=wk_t[:, 1], in_=d_wk[:, 1])
                for k2 in range(NK2):
                    nc.sync.dma_start(out=xfm[:, k2], in_=d_x_fm8[:, k2])
                nc.scalar.dma_start(out=wk_t[:, 2], in_=d_wk[:, 2])
                nc.scalar.dma_start(out=wk_t[:, 3], in_=d_wk[:, 3])
                for i in range(2, NT):
                    nc.scalar.dma_start(out=xts[i],
                                        in_=d_x_tm[i * 128:(i + 1) * 128, :])
                nc.sync.dma_start(out=scol, in_=d_scol[:])
                nc.sync.dma_start(out=ccol, in_=d_ccol[:])
                nc.sync.dma_start(out=sbt, in_=d_sb[:])
                nc.sync.dma_start(out=cbt, in_=d_cb[:])
                nc.vector.memset(eps1, EPS_LN)
                nc.vector.memset(v_aug[:, :, :, 64:65], 8.0)
                wv_t = wp.tile([128, NK2, 2, E], F8, tag='W')
                for k2 in range(NK2):
                    nc.gpsimd.dma_start(out=wv_t[:, k2], in_=d_wv[:, k2])

                # ---- Phase A: LN1 per token tile (vector stats, scalar app) ----
                with tc.tile_pool(name='ln1', bufs=4) as ap:
                    for i in range(NT):
                        xt = xts[i]
                        st = ap.tile([128, 2, 6], FP32, tag='st')
                        xg = xt[:].rearrange('p (g d) -> p g d', g=2)
                        nc.vector.bn_stats(out=st[:, 0, :], in_=xg[:, 0, :])
                        nc.vector.bn_stats(out=st[:, 1, :], in_=xg[:, 1, :])
                        mv = ap.tile([128, 2], FP32, tag='mv')
                        nc.vector.bn_aggr(out=mv, in_=st)
                        rstd = ap.tile([128, 1], FP32, tag='rstd')
                        nc.scalar.activation(out=rstd, in_=mv[:, 1:2], func=AF.Sqrt,
                                             bias=eps1, scale=1.0)
                        nc.vector.reciprocal(out=rstd, in_=rstd)
                        # -mu (bf16) for the x_c broadcast path
                        nc.vector.tensor_scalar(out=negmu_cols[:, i:i + 1],
                                                in0=mv[:, 0:1], scalar1=-1.0,
                                                scalar2=None, op0=AluOpType.mult)
                        if general_q:
                            nc.vector.tensor_copy(out=rst_cols[:, i:i + 1],
                                                  in_=rstd)
                        # qn = (x - mu) * rstd (+affine) on gpsimd (SBUF only)
                        if has_g1b1:
                            qtmp = ap.tile([128, E], FP32, tag='qtmp')
                            nc.gpsimd.tensor_scalar(out=qtmp, in0=xt,
                                                    scalar1=mv[:, 0:1],
                                                    scalar2=rstd,
                                                    op0=AluOpType.subtract,
                                                    op1=AluOpType.mult)
                            nc.vector.scalar_tensor_tensor(
                                out=qn[:, i, :], in0=qtmp, scalar=1.0, in1=g1b,
                                op0=AluOpType.mult, op1=AluOpType.mult)
                            nc.vector.tensor_tensor(out=qn[:, i, :],
                                                    in0=qn[:, i, :], in1=b1b,
                                                    op=AluOpType.add)
                        else:
                            nc.gpsimd.tensor_scalar(out=qn[:, i, :], in0=xt,
                                                    scalar1=mv[:, 0:1],
                                                    scalar2=rstd,
                                                    op0=AluOpType.subtract,
                                                    op1=AluOpType.mult)
                        # gather -mu into a flat [1, T] row (cross-partition DMA)
                        nc.sync.dma_start(
                            out=mu_row[:, i * 128:(i + 1) * 128],
                            in_=negmu_cols[:, i:i + 1])
                        if general_q:
                            nc.sync.dma_start(
                                out=r_row[:, i * 128:(i + 1) * 128],
                                in_=rst_cols[:, i:i + 1])

                # mu flat row -> partition broadcast -> centered x (fp8)
                nc.gpsimd.partition_broadcast(mu_bc, mu_row, channels=128)
                for k2 in range(NK2):
                    for h in range(2):
                        nc.gpsimd.tensor_tensor(out=xc8[:, k2, h, :],
                                                in0=xfm[:, k2, h, :],
                                                in1=mu_bc, op=AluOpType.add)
                if general_q:
                    nc.gpsimd.partition_broadcast(r_bc, r_row, channels=128)

                # ---- Phases B1/B2: K and V projections (fp8 DR) ----
                with tc.tile_pool(name='psB', bufs=8, space='PSUM') as psb:
                    def phm_tok_major(w_t, epilogue, nm):
                        for half in range(2):
                            ptiles = {}
                            for i in range(4 * half, 4 * half + 4):
                                for ch in range(2):
                                    pt = psb.tile([128, 512], FP32, tag='psB',
                                                  name=f'pb{nm}_{i}_{ch}')
                                    ptiles[i, ch] = pt
                            for k2 in range(NK2):
                                for i in range(4 * half, 4 * half + 4):
                                    isl = slice(i * 128, (i + 1) * 128)
                                    for ch in range(2):
                                        csl = slice(ch * 512, (ch + 1) * 512)
                                        nc.tensor.matmul(
                                            ptiles[i, ch],
                                            lhsT=xfm[:, k2, :, isl],
                                            rhs=w_t[:, k2, :, csl],
                                            perf_mode=DR,
                                            start=(k2 == 0), stop=(k2 == NK2 - 1))
                            for i in range(4 * half, 4 * half + 4):
                                for ch in range(2):
                                    epilogue(i, ch, ptiles[i, ch])

                    def k_epilogue(i, ch, pk):
                        csl = slice(ch * 512, (ch + 1) * 512)
                        if has_kb:
                            nc.vector.tensor_tensor(out=pk, in0=pk,
                                                    in1=kbb[:, csl],
                                                    op=AluOpType.add)
                        pkv = pk[:].rearrange('p (h d) -> p h d', d=64)
                        hsl = slice(ch * 8, (ch + 1) * 8)
                        # sin half on scalar engine, cos half on vector
                        nc.scalar.activation(
                            out=ksc[:, i, hsl, 0:64],
                            in_=pk[:].rearrange('p (h d) -> p h d', d=64),
                            func=AF.Relu, scale=scol[:, i:i + 1])
                        nc.vector.tensor_scalar(
                            out=ksc[:, i, hsl, 64:128], in0=pkv,
                            scalar1=0.0, scalar2=ccol[:, i:i + 1],
                            op0=AluOpType.max, op1=AluOpType.mult)

                    def v_epilogue(i, ch, pv):
                        csl = slice(ch * 512, (ch + 1) * 512)
                        if has_vb:
                            nc.vector.tensor_tensor(out=pv, in0=pv,
                                                    in1=vbb[:, csl],
                                                    op=AluOpType.add)
                        hsl = slice(ch * 8, (ch + 1) * 8)
                        nc.vector.tensor_scalar(
                            out=v_aug[:, i, hsl, 0:64],
                            in0=pv[:].rearrange('p (h d) -> p h d', d=64),
                            scalar1=0.125, scalar2=None, op0=AluOpType.mult)

                    phm_tok_major(wk_t, k_epilogue, 'k')
                    phm_tok_major(wv_t, v_epilogue, 'v')

                # Wq load (wk slot frees): gpsimd queue (scalar is busy with
                # epilogue ACTs; gpsimd is idle here)
                wq_t = wp.tile([128, NK2, 2, E], F8, tag='W')
                for k2 in range(NK2):
                    nc.gpsimd.dma_start(out=wq_t[:, k2], in_=d_wq[:, k2])

                # ---- Phase C: per-head kv partials (fp8 DR) + AllReduce ----
                with tc.tile_pool(name='psC', bufs=8, space='PSUM') as psc:
                    for h in range(H):
                        pkv = psc.tile([128, 65], FP32, tag='psC', name=f'kv{h}')
                        for i2 in range(NT // 2):
                            nc.tensor.matmul(
                                pkv,
                                lhsT=ksc[:, 2 * i2:2 * i2 + 2, h, :],
                                rhs=v_aug[:, 2 * i2:2 * i2 + 2, h, :],
                                perf_mode=DR,
                                start=(i2 == 0), stop=(i2 == NT // 2 - 1))
                        nc.vector.tensor_scalar(
                            out=kvp[:, h * 65:(h + 1) * 65], in0=pkv,
                            scalar1=IWS, scalar2=None, op0=AluOpType.mult)

                nc.gpsimd.dma_start(out=kv_cc_in[:], in_=kvp)
                nc.gpsimd.collective_compute(
                    'AllReduce', AluOpType.add,
                    ins=[kv_cc_in.opt()], outs=[kv_cc_out.opt()],
                    replica_groups=RG)

            # Wo load (wv slot frees): gpsimd queue, during the collective
            wo_t = wp.tile([128, NK2, 2, E], F8, tag='W')
            for k2 in range(NK2):
                nc.gpsimd.dma_start(out=wo_t[:, k2], in_=d_wo[:, k2])
            nc.gpsimd.dma_start(out=kvb, in_=kv_cc_out[:])

            # ---- Phase D: Q projection (fp8 DR from x_c) ----
            with (
                tc.tile_pool(name='psD', bufs=4, space='PSUM') as psd,
                tc.tile_pool(name='qsb', bufs=4) as qsb,
            ):
                for ch in range(2):
                    csl = slice(ch * 512, (ch + 1) * 512)
                    for j in range(NJ):
                        jsl = slice(j * 128, (j + 1) * 128)
                        pq = psd.tile([128, 512], FP32, tag='psD',
                                      name=f'pq{ch}_{j}')
                        for k2 in range(NK2):
                            nc.tensor.matmul(
                                pq, lhsT=wq_t[:, k2, :, jsl],
                                rhs=xc8[:, k2, :, csl],
                                perf_mode=DR,
                                start=(k2 == 0), stop=(k2 == NK2 - 1))
                        if general_q:
                            nc.vector.tensor_tensor(out=pq, in0=pq,
                                                    in1=r_bc[:, csl],
                                                    op=AluOpType.mult)
                        qrel = qsb.tile([128, 512], BF, tag='qrel')
                        if general_q:
                            nc.scalar.activation(out=qrel, in_=pq, func=AF.Relu,
                                                 bias=qcj[:, j:j + 1])
                        else:
                            nc.scalar.activation(out=qrel, in_=pq, func=AF.Relu)
                        for hh in range(2):
                            h = 2 * j + hh
                            rs = slice(hh * 64, (hh + 1) * 64)
                            nc.vector.tensor_tensor(
                                out=qq[0:64, h, csl], in0=qrel[rs, :],
                                in1=sbt[rs, csl], op=AluOpType.mult)
                            nc.vector.tensor_tensor(
                                out=qq[64:128, h, csl], in0=qrel[rs, :],
                                in1=cbt[rs, csl], op=AluOpType.mult)

            # ---- Phases E/F/G interleaved ----
            with (
                tc.tile_pool(name='ef', bufs=3) as efp,
                tc.tile_pool(name='psE', bufs=4, space='PSUM') as pse,
                tc.tile_pool(name='go', bufs=4) as gop,
                tc.tile_pool(name='psG', bufs=4, space='PSUM') as psg,
            ):
                def emit_attn_ln2(i):
                    rsl = slice(i * 128, (i + 1) * 128)
                    yt = efp.tile([128, H, 64], BF, tag='yt')
                    z16 = efp.tile([128, H], FP32, tag='z16')
                    pas = []
                    for g in range(4):
                        pa = pse.tile([128, 4 * 65], FP32, tag='psE',
                                      name=f'pa_{i}_{g}')
                        pas.append(pa)
                        for hh in range(4):
                            h = 4 * g + hh
                            nc.tensor.matmul(pa[:, hh * 65:(hh + 1) * 65],
                                             lhsT=qq[:, h, rsl],
                                             rhs=kvb[:, h * 65:(h + 1) * 65],
                                             start=True, stop=True)
                        pav = pa[:].rearrange('p (h c) -> p h c', c=65)
                        nc.vector.tensor_scalar(
                            out=z16[:, g * 4:(g + 1) * 4], in0=pav[:, :, 64],
                            scalar1=EPS_ATTN, scalar2=None, op0=AluOpType.max)
                    nc.vector.reciprocal(out=z16, in_=z16)
                    for g in range(4):
                        pav = pas[g][:].rearrange('p (h c) -> p h c', c=65)
                        zb = z16[:, g * 4:(g + 1) * 4].broadcast_to((128, 4, 64))
                        nc.vector.tensor_tensor(out=yt[:, g * 4:(g + 1) * 4, :],
                                                in0=pav[:, :, 0:64], in1=zb,
                                                op=AluOpType.mult)
                    ytf = yt[:].rearrange('p h d -> p (h d)')
                    # residual + LN2 stats via accumulating side-outputs
                    s1 = efp.tile([128, 1], FP32, tag='s1')
                    s2 = efp.tile([128, 1], FP32, tag='s2')
                    ysq = efp.tile([128, E], BF, tag='ysq')
                    nc.vector.scalar_tensor_tensor(
                        out=ytf, in0=ytf, scalar=1.0, in1=qn[:, i, :],
                        op0=AluOpType.mult, op1=AluOpType.add, accum_out=s1)
                    nc.vector.scalar_tensor_tensor(
                        out=ysq, in0=ytf, scalar=1.0, in1=ytf,
                        op0=AluOpType.mult, op1=AluOpType.mult, accum_out=s2)
                    mu2c = efp.tile([128, 1], FP32, tag='mu2c')
                    varc = efp.tile([128, 1], FP32, tag='varc')
                    nc.vector.tensor_scalar(out=mu2c, in0=s1, scalar1=1.0 / E,
                                            scalar2=None, op0=AluOpType.mult)
                    nc.vector.tensor_scalar(out=varc, in0=s2, scalar1=1.0 / E,
                                            scalar2=None, op0=AluOpType.mult)
                    m2s = efp.tile([128, 1], FP32, tag='m2s')
                    nc.vector.tensor_scalar(out=m2s, in0=mu2c, scalar1=mu2c,
                                            scalar2=None, op0=AluOpType.mult)
                    nc.vector.tensor_tensor(out=varc, in0=varc, in1=m2s,
                                            op=AluOpType.subtract)
                    rstd2 = efp.tile([128, 1], FP32, tag='rstd2')
                    nc.scalar.activation(out=rstd2, in_=varc, func=AF.Sqrt,
                                         bias=eps1, scale=1.0)
                    nc.vector.reciprocal(out=rstd2, in_=rstd2)
                    nb2 = efp.tile([128, 1], FP32, tag='nb2')
                    nc.vector.tensor_scalar(out=nb2, in0=mu2c,
                                            scalar1=rstd2, scalar2=-1.0,
                                            op0=AluOpType.mult,
                                            op1=AluOpType.mult)
                    nc.scalar.activation(out=xh[:, i, :], in_=ytf, func=AF.Identity,
                                         bias=nb2, scale=rstd2)
                    nc.gpsimd.dma_start(out=xh_dram[rsl, :], in_=xh[:, i, :])

                def emit_xh_transpose(tsl, jlo, jhi):
                    for j in range(jlo, jhi):
                        qeng = nc.sync if j % 2 else nc.scalar
                        qeng.dma_start(out=xhT[:, j, tsl],
                                       in_=xh_dram[tsl, j * 128:(j + 1) * 128],
                                       transpose=True)
                    for j in range(jlo, jhi):
                        nc.gpsimd.tensor_copy(
                            out=xhT8[:, j // 2, j % 2, tsl],
                            in_=xhT[:, j, tsl])

                def emit_o(i):
                    isl = slice(i * 128, (i + 1) * 128)
                    for ch in range(2):
                        csl = slice(ch * 512, (ch + 1) * 512)
                        po = psg.tile([128, 512], FP32, tag='psG',
                                      name=f'po_{i}_{ch}')
                        for k2 in range(NK2):
                            nc.tensor.matmul(
                                po, lhsT=xhT8[:, k2, :, isl],
                                rhs=wo_t[:, k2, :, csl],
                                perf_mode=DR,
                                start=(k2 == 0), stop=(k2 == NK2 - 1))
                        ot = gop.tile([128, 512], BF, tag='ot')
                        xres = xh[:, i, csl]
                        if has_g2:
                            xg2 = gop.tile([128, 512], FP32, tag='xg2')
                            nc.vector.tensor_tensor(out=xg2, in0=xh[:, i, csl],
                                                    in1=g2b[:, csl],
                                                    op=AluOpType.mult)
                            xres = xg2
                        if ch == 0:
                            nc.vector.scalar_tensor_tensor(
                                out=ot, in0=po, scalar=IWS, in1=xres,
                                op0=AluOpType.mult, op1=AluOpType.add)
                        else:
                            # off the vector engine: scale on scalar, add on pool
                            ot1 = gop.tile([128, 512], BF, tag='ot1')
                            nc.scalar.activation(out=ot1, in_=po, func=AF.Identity,
                                                 scale=IWS)
                            nc.gpsimd.tensor_tensor(out=ot, in0=ot1, in1=xres,
                                                    op=AluOpType.add)
                        if has_b2o:
                            nc.vector.tensor_tensor(out=ot, in0=ot,
                                                    in1=b2ob[:, csl],
                                                    op=AluOpType.add)
                        qeng = nc.scalar if ch else nc.sync
                        qeng.dma_start(out=d_out[isl, csl], in_=ot)

                emit_attn_ln2(0)
                emit_attn_ln2(1)
                emit_attn_ln2(2)
                emit_attn_ln2(3)
                emit_xh_transpose(slice(0, 512), 0, NJ)
                emit_attn_ln2(4)
                emit_o(0)
                emit_attn_ln2(5)
                emit_o(1)
                emit_attn_ln2(6)
                emit_o(2)
                emit_attn_ln2(7)
                emit_o(3)
                emit_xh_transpose(slice(512, 1024), 0, NJ)
                emit_o(4)
                emit_o(5)
                emit_o(6)
                emit_o(7)

    nc.compile()
    return nc


def _get_program(flags):
    if flags not in _BUILD_CACHE:
        _BUILD_CACHE[flags] = _build_program(flags)
    return _BUILD_CACHE[flags]


def _phm_weight(A, S):
    f = A.shape[0]
    din, dout = f * S.shape[1], f * S.shape[2]
    W = np.einsum('nij,nkl->ikjl', np.asarray(A, np.float32), np.asarray(S, np.float32))
    return np.ascontiguousarray(W.reshape(din, dout))


def _w8(W):
    """[E, E] fp32 -> [128, NK2, 2, E] fp8 with x64 scale."""
    Wv = (W * WS).reshape(NK2, 2, 128, E)
    return np.ascontiguousarray(np.transpose(Wv, (2, 0, 1, 3))).astype(E4M3)


def kernel(**inputs):
    query = np.asarray(inputs['query'], np.float32)
    g1 = np.asarray(inputs['g1'], np.float32)
    b1 = np.asarray(inputs['b1'], np.float32)
    g2 = np.asarray(inputs['g2'], np.float32)
    b2 = np.asarray(inputs['b2'], np.float32)
    qb = np.asarray(inputs['qb'], np.float32)
    kb = np.asarray(inputs['kb'], np.float32)
    vb = np.asarray(inputs['vb'], np.float32)
    ob = np.asarray(inputs['ob'], np.float32)

    Wq = _phm_weight(inputs['qA'], inputs['qS'])
    Wk = _phm_weight(inputs['kA'], inputs['kS'])
    Wv = _phm_weight(inputs['vA'], inputs['vS'])
    Wo = _phm_weight(inputs['oA'], inputs['oS'])

    has_g1b1 = not (np.all(g1 == 1.0) and np.all(b1 == 0.0))
    has_qb = bool(np.any(qb != 0.0))
    has_kb = bool(np.any(kb != 0.0))
    has_vb = bool(np.any(vb != 0.0))
    has_g2 = not np.all(g2 == 1.0)
    # final = xh@(g2*Wo) + xh*g2 + C,  C = b2@Wo + ob + b2
    C = b2 @ Wo + ob + b2
    has_b2o = bool(np.any(C != 0.0))
    general_q = has_g1b1 or has_qb
    flags = (has_g1b1, has_qb, has_kb, has_vb, has_g2, has_b2o)

    nc = _get_program(flags)

    Wg = g2[:, None] * Wo
    Wq_eff = g1[:, None] * Wq if has_g1b1 else Wq
    wq_b = _w8(Wq_eff)
    wk_b = _w8(Wk)
    wv_b = _w8(Wv)
    wo_b = _w8(Wg)

    s_full = np.sin((np.pi / 2) * np.arange(1, L + 1, dtype=np.float32) / L)
    c_full = np.cos((np.pi / 2) * np.arange(1, L + 1, dtype=np.float32) / L)

    in_maps = []
    for core in range(NCORES):
        b = core // 2
        l0 = (core % 2) * T
        x = np.ascontiguousarray(query[l0:l0 + T, b, :])
        xT = np.transpose(x).reshape(NK2, 2, 128, T)
        s = s_full[l0:l0 + T]
        c = c_full[l0:l0 + T]
        im = {
            'x_tm': x.astype(BF16),
            'x_fm8': np.ascontiguousarray(np.transpose(xT, (2, 0, 1, 3))).astype(E4M3),
            'wq8': wq_b, 'wk8': wk_b, 'wv8': wv_b, 'wo8': wo_b,
            's_bcast': np.ascontiguousarray(
                np.broadcast_to(s / WS, (128, T))).astype(BF16),
            'c_bcast': np.ascontiguousarray(
                np.broadcast_to(c / WS, (128, T))).astype(BF16),
            's_cols': np.ascontiguousarray((s / 8.0).reshape(NT, 128).T),
            'c_cols': np.ascontiguousarray((c / 8.0).reshape(NT, 128).T),
        }
        if has_g1b1:
            im['g1_b'] = np.ascontiguousarray(np.broadcast_to(g1, (128, E)))
            im['b1_b'] = np.ascontiguousarray(np.broadcast_to(b1, (128, E)))
        if general_q:
            cj = ((b1 @ Wq if has_g1b1 else np.zeros(E, np.float32)) + qb) * WS
            im['q_cj'] = np.ascontiguousarray(cj.reshape(NJ, 128).T)
        if has_kb:
            im['kb_b'] = np.ascontiguousarray(np.broadcast_to(kb * WS, (128, E)))
        if has_vb:
            im['vb_b'] = np.ascontiguousarray(np.broadcast_to(vb * WS, (128, E)))
        if has_g2:
            im['g2_b'] = np.ascontiguousarray(np.broadcast_to(g2, (128, E)))
        if has_b2o:
            im['b2o_b'] = np.ascontiguousarray(np.broadcast_to(C, (128, E)))
        in_maps.append(im)

    trace = bool(os.environ.get('KERNEL_TRACE'))
    res = run_bass_kernel_spmd(nc, in_maps, list(range(NCORES)), trace=trace)
    kernel._last_exec_ns = res.exec_time_ns

    out = np.empty((L, N, E), np.float32)
    for core in range(NCORES):
        b = core // 2
        l0 = (core % 2) * T
        out[l0:l0 + T, b, :] = res.results[core]['out'].astype(np.float32)
    return out


kernel._last_exec_ns = None


# revision 21
# speedup vs baseline: 1.6175x; 1.6175x over previous
"""Cosformer attention Bass kernel for 8 trn2 NeuronCores — fp8 edition.

Sharding: core c handles batch c//2, sequence half c%2 (1024 tokens).
Per-head linear-attention state (kv, ksum) is AllReduce'd (bf16) between
the two cores sharing a batch.

The 4 big projection matmuls run in fp8e4m3 with DoubleRow perf mode
(K=256 per instruction, ~2x bf16 rate). Weights are scaled x64 on host to
escape the fp8 subnormal zone; the scales cancel through the cosformer
z-normalization or are folded into epilogues.

LN1 folding: rstd cancels through z-normalization (relu commutes with
positive per-token scale), so the Q projection consumes centered x only:
x_c = x - mu, built feature-major from a partition-broadcast of mu. This
removes the qn transposes entirely; qn stays SBUF-resident for the
residual.

The final residual (+ out) is applied as an epilogue add (NOT folded into
Wo: the identity diagonal would quantize at ~6% in fp8).
"""

import sys

for _p in ('/opt/trn_rl_repo',):
    if _p not in sys.path:
        sys.path.insert(0, _p)

import importlib.util as _ilu
import os

os.environ.setdefault('NEURON_RT_RESET_CORES', '1')

# The image's antenv lacks axon_hooks (needed for trace=True); register ours.
if 'antenv.axon_hooks' not in sys.modules:
    _hp = '/opt/trn_rl_repo/antenv/axon_hooks.py'
    if os.path.exists(_hp):
        _spec = _ilu.spec_from_file_location('antenv.axon_hooks', _hp)
        _mod = _ilu.module_from_spec(_spec)
        _spec.loader.exec_module(_mod)
        sys.modules['antenv.axon_hooks'] = _mod
    else:
        import types as _types

        _mod = _types.ModuleType('antenv.axon_hooks')
        _mod._hook = None
        _mod.set_axon_ntff_profile_hook = lambda h: setattr(_mod, '_hook', h)
        _mod.get_axon_ntff_profile_hook = lambda: _mod._hook
        sys.modules['antenv.axon_hooks'] = _mod


def _register_ntff_hook():
    """If boot didn't register the NTFF profile hook (image antenv lacks
    axon_hooks), drive NRT profiling via ctypes into libaxon_pjrt.so."""
    import contextlib
    import ctypes

    mod = sys.modules['antenv.axon_hooks']
    if mod.get_axon_ntff_profile_hook() is not None:
        return
    so_path = '/opt/axon/libaxon_pjrt.so'
    if not os.path.exists(so_path):
        return
    try:
        lib = ctypes.CDLL(so_path)
        if not hasattr(lib, 'axon_start_nrt_profile'):
            return
        lib.axon_start_nrt_profile.argtypes = [
            ctypes.POINTER(ctypes.c_int64), ctypes.c_size_t]
        lib.axon_start_nrt_profile.restype = ctypes.c_int64
        lib.axon_stop_nrt_profile.argtypes = [ctypes.c_char_p]
        lib.axon_stop_nrt_profile.restype = ctypes.c_int64
    except OSError:
        return

    @contextlib.contextmanager
    def _hook(output_dir, device_ids):
        import jax
        jax.devices()
        if device_ids:
            ids = (ctypes.c_int64 * len(device_ids))(*device_ids)
            rc = lib.axon_start_nrt_profile(ids, len(device_ids))
        else:
            rc = lib.axon_start_nrt_profile(None, 0)
        if rc != 0:
            raise RuntimeError(f'axon_start_nrt_profile rc={rc}')
        try:
            yield
        finally:
            n = lib.axon_stop_nrt_profile(str(output_dir).encode())
            if n < 0:
                raise RuntimeError(f'axon_stop_nrt_profile rc={n}')

    mod.set_axon_ntff_profile_hook(_hook)


_register_ntff_hook()

import numpy as np
import ml_dtypes

import concourse.bass as bass
import concourse.tile as tile
from concourse import bacc, mybir
from concourse.alu_op_type import AluOpType
from concourse.bass_utils import run_bass_kernel_spmd

BF16 = ml_dtypes.bfloat16
E4M3 = ml_dtypes.float8_e4m3
FP32 = mybir.dt.float32
BF = mybir.dt.bfloat16
F8 = mybir.dt.float8e4
AF = mybir.ActivationFunctionType
DR = mybir.MatmulPerfMode.DoubleRow

L, N, E, H, D = 2048, 4, 1024, 16, 64
T = 1024            # tokens per core
NT = T // 128       # 8 token tiles
NK2 = 4             # DoubleRow contraction steps (256 features each)
NJ = E // 128       # 8 output-feature tiles
NCORES = 8
EPS_LN = 1e-5
EPS_ATTN = 1e-6
WS = 64.0           # host weight scale
IWS = 1.0 / WS

_BUILD_CACHE = {}


def _build_program(flags):
    """flags: (has_g1b1, has_qb, has_kb, has_vb, has_g2, has_b2o)."""
    has_g1b1, has_qb, has_kb, has_vb, has_g2, has_b2o = flags
    general_q = has_g1b1 or has_qb   # r no longer cancels for the Q path

    nc = bacc.Bacc("TRN2", target_bir_lowering=False, debug=False,
                   num_devices=NCORES)

    # ---- DRAM I/O ----
    d_x_tm = nc.dram_tensor('x_tm', [T, E], BF, kind='ExternalInput')
    d_x_fm8 = nc.dram_tensor('x_fm8', [128, NK2, 2, T], F8, kind='ExternalInput')
    d_wq = nc.dram_tensor('wq8', [128, NK2, 2, E], F8, kind='ExternalInput')
    d_wk = nc.dram_tensor('wk8', [128, NK2, 2, E], F8, kind='ExternalInput')
    d_wv = nc.dram_tensor('wv8', [128, NK2, 2, E], F8, kind='ExternalInput')
    d_wo = nc.dram_tensor('wo8', [128, NK2, 2, E], F8, kind='ExternalInput')
    d_sb = nc.dram_tensor('s_bcast', [128, T], BF, kind='ExternalInput')
    d_cb = nc.dram_tensor('c_bcast', [128, T], BF, kind='ExternalInput')
    d_scol = nc.dram_tensor('s_cols', [128, NT], FP32, kind='ExternalInput')
    d_ccol = nc.dram_tensor('c_cols', [128, NT], FP32, kind='ExternalInput')
    d_g1b = nc.dram_tensor('g1_b', [128, E], FP32, kind='ExternalInput') if has_g1b1 else None
    d_b1b = nc.dram_tensor('b1_b', [128, E], FP32, kind='ExternalInput') if has_g1b1 else None
    d_qcj = nc.dram_tensor('q_cj', [128, NJ], FP32, kind='ExternalInput') if general_q else None
    d_kbb = nc.dram_tensor('kb_b', [128, E], FP32, kind='ExternalInput') if has_kb else None
    d_vbb = nc.dram_tensor('vb_b', [128, E], FP32, kind='ExternalInput') if has_vb else None
    d_g2b = nc.dram_tensor('g2_b', [128, E], FP32, kind='ExternalInput') if has_g2 else None
    d_b2ob = nc.dram_tensor('b2o_b', [128, E], FP32, kind='ExternalInput') if has_b2o else None
    d_out = nc.dram_tensor('out', [T, E], BF, kind='ExternalOutput')

    RG = [[0, 1], [2, 3], [4, 5], [6, 7]]

    with tile.TileContext(nc) as tc:
        with (
            tc.tile_pool(name='persist', bufs=1) as pp,
            tc.tile_pool(name='wpool', bufs=2) as wp,
            tc.tile_pool(name='dram', bufs=1, space='DRAM') as dp,
        ):
            # ---- persistent tiles ----
            sbt = pp.tile([128, T], BF, tag='sbt')          # s/WS bcast
            cbt = pp.tile([128, T], BF, tag='cbt')
            scol = pp.tile([128, NT], FP32, tag='scol')     # s/8 cols
            ccol = pp.tile([128, NT], FP32, tag='ccol')
            eps1 = pp.tile([128, 1], FP32, tag='eps1')
            qn = pp.tile([128, NT, E], BF, tag='qn')        # LN1 out, token-major
            xh = pp.tile([128, NT, E], BF, tag='xh')        # LN2 out, token-major
            xc8 = pp.tile([128, NK2, 2, T], F8, tag='xc8')  # centered x, fm
            ksc = pp.tile([128, NT, H, 128], F8, tag='ksc') # 8*k*[s|c], tok-major
            v_aug = pp.tile([128, NT, H, 65], F8, tag='vaug')
            qq = pp.tile([128, H, T], BF, tag='qq')         # q~*[s|c] per head, fm
            kvb = pp.tile([128, H * 65], BF, tag='kvb')     # reduced kv (true scale)
            kvp = pp.tile([128, H * 65], BF, tag='kvp')
            xhT = pp.tile([128, NJ, T], BF, tag='xhT')      # transposed xh
            xhT8 = pp.tile([128, NK2, 2, T], F8, tag='xhT8')
            negmu_cols = pp.tile([128, NT], BF, tag='nmu')  # -mu per token
            mu_row = pp.tile([1, T], BF, tag='murow')
            mu_bc = pp.tile([128, T], BF, tag='mubc')
            rst_cols = pp.tile([128, NT], FP32, tag='rst') if general_q else None
            r_row = pp.tile([1, T], FP32, tag='rrow') if general_q else None
            r_bc = pp.tile([128, T], FP32, tag='rbc') if general_q else None

            g1b = b1b = qcj = kbb = vbb = g2b = b2ob = None
            if has_g1b1:
                g1b = pp.tile([128, E], FP32, tag='g1b')
                b1b = pp.tile([128, E], FP32, tag='b1b')
                nc.gpsimd.dma_start(out=g1b, in_=d_g1b[:])
                nc.gpsimd.dma_start(out=b1b, in_=d_b1b[:])
            if general_q:
                qcj = pp.tile([128, NJ], FP32, tag='qcj')
                nc.gpsimd.dma_start(out=qcj, in_=d_qcj[:])
            if has_kb:
                kbb = pp.tile([128, E], FP32, tag='kbb')
                nc.gpsimd.dma_start(out=kbb, in_=d_kbb[:])
            if has_vb:
                vbb = pp.tile([128, E], FP32, tag='vbb')
                nc.gpsimd.dma_start(out=vbb, in_=d_vbb[:])
            if has_g2:
                g2b = pp.tile([128, E], FP32, tag='g2b')
                nc.gpsimd.dma_start(out=g2b, in_=d_g2b[:])
            if has_b2o:
                b2ob = pp.tile([128, E], FP32, tag='b2ob')
                nc.gpsimd.dma_start(out=b2ob, in_=d_b2ob[:])

            # DRAM scratch
            xh_dram = dp.tile([T, E], BF)
            kv_cc_in = dp.tile([128, H * 65], BF)
            kv_cc_out = dp.tile([128, H * 65], BF)

            # ---- front-loaded DMAs ----
            # scalar queue: x_tm tiles (LN1 needs them first) interleaved
            # with Wk chunks (PE needs chunk 0 at ~1.5us)
            xfm = pp.tile([128, NK2, 2, T], F8, tag='xfm')
            wk_t = wp.tile([128, NK2, 2, E], F8, tag='W')
            with tc.tile_pool(name='xtiles', bufs=8) as xp:
                xts = []
                for i in range(NT):
                    xt = xp.tile([128, E], BF, tag='xt', name=f'xt{i}')
                    xts.append(xt)
                nc.scalar.dma_start(out=xts[0], in_=d_x_tm[0:128, :])
                nc.scalar.dma_start(out=wk_t[:, 0], in_=d_wk[:, 0])
                nc.scalar.dma_start(out=xts[1], in_=d_x_tm[128:256, :])
                nc.scalar.dma_start(out=wk_t[:, 1], in_=d_wk[:, 1])
                for k2 in range(NK2):
                    nc.sync.dma_start(out=xfm[:, k2], in_=d_x_fm8[:, k2])
                nc.scalar.dma_start(out=wk_t[:, 2], in_=d_wk[:, 2])
                nc.scalar.dma_start(out=wk_t[:, 3], in_=d_wk[:, 3])
                for i in range(2, NT):
                    nc.scalar.dma_start(out=xts[i],
                                        in_=d_x_tm[i * 128:(i + 1) * 128, :])
                nc.sync.dma_start(out=scol, in_=d_scol[:])
                nc.sync.dma_start(out=ccol, in_=d_ccol[:])
                nc.sync.dma_start(out=sbt, in_=d_sb[:])
                nc.sync.dma_start(out=cbt, in_=d_cb[:])
                nc.vector.memset(eps1, EPS_LN)
                nc.vector.memset(v_aug[:, :, :, 64:65], 8.0)
                wv_t = wp.tile([128, NK2, 2, E], F8, tag='W')
                for k2 in range(NK2):
                    nc.gpsimd.dma_start(out=wv_t[:, k2], in_=d_wv[:, k2])

                # ---- Phase A: LN1 per token tile (vector stats, scalar app) ----
                with tc.tile_pool(name='ln1', bufs=4) as ap:
                    for i in range(NT):
                        xt = xts[i]
                        st = ap.tile([128, 2, 6], FP32, tag='st')
                        xg = xt[:].rearrange('p (g d) -> p g d', g=2)
                        nc.vector.bn_stats(out=st[:, 0, :], in_=xg[:, 0, :])
                        nc.vector.bn_stats(out=st[:, 1, :], in_=xg[:, 1, :])
                        mv = ap.tile([128, 2], FP32, tag='mv')
                        nc.vector.bn_aggr(out=mv, in_=st)
                        rstd = ap.tile([128, 1], FP32, tag='rstd')
                        nc.scalar.activation(out=rstd, in_=mv[:, 1:2], func=AF.Sqrt,
                                             bias=eps1, scale=1.0)
                        nc.vector.reciprocal(out=rstd, in_=rstd)
                        # -mu (bf16) for the x_c broadcast path
                        nc.vector.tensor_scalar(out=negmu_cols[:, i:i + 1],
                                                in0=mv[:, 0:1], scalar1=-1.0,
                                                scalar2=None, op0=AluOpType.mult)
                        if general_q:
                            nc.vector.tensor_copy(out=rst_cols[:, i:i + 1],
                                                  in_=rstd)
                        # qn = (x - mu) * rstd (+affine) via scalar ACT
                        nbias = ap.tile([128, 1], FP32, tag='nb')
                        nc.vector.tensor_scalar(out=nbias, in0=mv[:, 0:1],
                                                scalar1=rstd, scalar2=-1.0,
                                                op0=AluOpType.mult,
                                                op1=AluOpType.mult)
                        if has_g1b1:
                            qtmp = ap.tile([128, E], FP32, tag='qtmp')
                            nc.scalar.activation(out=qtmp, in_=xt,
                                                 func=AF.Identity,
                                                 bias=nbias, scale=rstd)
                            nc.vector.scalar_tensor_tensor(
                                out=qn[:, i, :], in0=qtmp, scalar=1.0, in1=g1b,
                                op0=AluOpType.mult, op1=AluOpType.mult)
                            nc.vector.tensor_tensor(out=qn[:, i, :],
                                                    in0=qn[:, i, :], in1=b1b,
                                                    op=AluOpType.add)
                        else:
                            nc.scalar.activation(out=qn[:, i, :], in_=xt,
                                                 func=AF.Identity, bias=nbias,
                                                 scale=rstd)
                        # gather -mu into a flat [1, T] row (cross-partition DMA)
                        nc.sync.dma_start(
                            out=mu_row[:, i * 128:(i + 1) * 128],
                            in_=negmu_cols[:, i:i + 1])
                        if general_q:
                            nc.sync.dma_start(
                                out=r_row[:, i * 128:(i + 1) * 128],
                                in_=rst_cols[:, i:i + 1])

                # mu flat row -> partition broadcast -> centered x (fp8)
                nc.gpsimd.partition_broadcast(mu_bc, mu_row, channels=128)
                for k2 in range(NK2):
                    for h in range(2):
                        nc.vector.tensor_tensor(out=xc8[:, k2, h, :],
                                                in0=xfm[:, k2, h, :],
                                                in1=mu_bc, op=AluOpType.add)
                if general_q:
                    nc.gpsimd.partition_broadcast(r_bc, r_row, channels=128)

                # ---- Phases B1/B2: K and V projections (fp8 DR) ----
                with tc.tile_pool(name='psB', bufs=8, space='PSUM') as psb:
                    def phm_tok_major(w_t, epilogue, nm):
                        for half in range(2):
                            ptiles = {}
                            for i in range(4 * half, 4 * half + 4):
                                for ch in range(2):
                                    pt = psb.tile([128, 512], FP32, tag='psB',
                                                  name=f'pb{nm}_{i}_{ch}')
                                    ptiles[i, ch] = pt
                            for k2 in range(NK2):
                                for i in range(4 * half, 4 * half + 4):
                                    isl = slice(i * 128, (i + 1) * 128)
                                    for ch in range(2):
                                        csl = slice(ch * 512, (ch + 1) * 512)
                                        nc.tensor.matmul(
                                            ptiles[i, ch],
                                            lhsT=xfm[:, k2, :, isl],
                                            rhs=w_t[:, k2, :, csl],
                                            perf_mode=DR,
                                            start=(k2 == 0), stop=(k2 == NK2 - 1))
                            for i in range(4 * half, 4 * half + 4):
                                for ch in range(2):
                                    epilogue(i, ch, ptiles[i, ch])

                    def k_epilogue(i, ch, pk):
                        csl = slice(ch * 512, (ch + 1) * 512)
                        if has_kb:
                            nc.vector.tensor_tensor(out=pk, in0=pk,
                                                    in1=kbb[:, csl],
                                                    op=AluOpType.add)
                        pkv = pk[:].rearrange('p (h d) -> p h d', d=64)
                        hsl = slice(ch * 8, (ch + 1) * 8)
                        # sin half on scalar engine, cos half on vector
                        nc.scalar.activation(
                            out=ksc[:, i, hsl, 0:64],
                            in_=pk[:].rearrange('p (h d) -> p h d', d=64),
                            func=AF.Relu, scale=scol[:, i:i + 1])
                        nc.vector.tensor_scalar(
                            out=ksc[:, i, hsl, 64:128], in0=pkv,
                            scalar1=0.0, scalar2=ccol[:, i:i + 1],
                            op0=AluOpType.max, op1=AluOpType.mult)

                    def v_epilogue(i, ch, pv):
                        csl = slice(ch * 512, (ch + 1) * 512)
                        if has_vb:
                            nc.vector.tensor_tensor(out=pv, in0=pv,
                                                    in1=vbb[:, csl],
                                                    op=AluOpType.add)
                        hsl = slice(ch * 8, (ch + 1) * 8)
                        nc.vector.tensor_scalar(
                            out=v_aug[:, i, hsl, 0:64],
                            in0=pv[:].rearrange('p (h d) -> p h d', d=64),
                            scalar1=0.125, scalar2=None, op0=AluOpType.mult)

                    phm_tok_major(wk_t, k_epilogue, 'k')
                    phm_tok_major(wv_t, v_epilogue, 'v')

                # Wq load (wk slot frees): gpsimd queue (scalar is busy with
                # epilogue ACTs; gpsimd is idle here)
                wq_t = wp.tile([128, NK2, 2, E], F8, tag='W')
                for k2 in range(NK2):
                    nc.gpsimd.dma_start(out=wq_t[:, k2], in_=d_wq[:, k2])

                # ---- Phase C: per-head kv partials (fp8 DR) + AllReduce ----
                with tc.tile_pool(name='psC', bufs=8, space='PSUM') as psc:
                    for h in range(H):
                        pkv = psc.tile([128, 65], FP32, tag='psC', name=f'kv{h}')
                        for i2 in range(NT // 2):
                            nc.tensor.matmul(
                                pkv,
                                lhsT=ksc[:, 2 * i2:2 * i2 + 2, h, :],
                                rhs=v_aug[:, 2 * i2:2 * i2 + 2, h, :],
                                perf_mode=DR,
                                start=(i2 == 0), stop=(i2 == NT // 2 - 1))
                        nc.vector.tensor_scalar(
                            out=kvp[:, h * 65:(h + 1) * 65], in0=pkv,
                            scalar1=IWS, scalar2=None, op0=AluOpType.mult)

                nc.gpsimd.dma_start(out=kv_cc_in[:], in_=kvp)
                nc.gpsimd.collective_compute(
                    'AllReduce', AluOpType.add,
                    ins=[kv_cc_in.opt()], outs=[kv_cc_out.opt()],
                    replica_groups=RG)

            # Wo load (wv slot frees): gpsimd queue, during the collective
            wo_t = wp.tile([128, NK2, 2, E], F8, tag='W')
            for k2 in range(NK2):
                nc.gpsimd.dma_start(out=wo_t[:, k2], in_=d_wo[:, k2])
            nc.gpsimd.dma_start(out=kvb, in_=kv_cc_out[:])

            # ---- Phase D: Q projection (fp8 DR from x_c) ----
            with (
                tc.tile_pool(name='psD', bufs=4, space='PSUM') as psd,
                tc.tile_pool(name='qsb', bufs=4) as qsb,
            ):
                for ch in range(2):
                    csl = slice(ch * 512, (ch + 1) * 512)
                    for j in range(NJ):
                        jsl = slice(j * 128, (j + 1) * 128)
                        pq = psd.tile([128, 512], FP32, tag='psD',
                                      name=f'pq{ch}_{j}')
                        for k2 in range(NK2):
                            nc.tensor.matmul(
                                pq, lhsT=wq_t[:, k2, :, jsl],
                                rhs=xc8[:, k2, :, csl],
                                perf_mode=DR,
                                start=(k2 == 0), stop=(k2 == NK2 - 1))
                        if general_q:
                            nc.vector.tensor_tensor(out=pq, in0=pq,
                                                    in1=r_bc[:, csl],
                                                    op=AluOpType.mult)
                        qrel = qsb.tile([128, 512], BF, tag='qrel')
                        if general_q:
                            nc.scalar.activation(out=qrel, in_=pq, func=AF.Relu,
                                                 bias=qcj[:, j:j + 1])
                        else:
                            nc.scalar.activation(out=qrel, in_=pq, func=AF.Relu)
                        for hh in range(2):
                            h = 2 * j + hh
                            rs = slice(hh * 64, (hh + 1) * 64)
                            nc.vector.tensor_tensor(
                                out=qq[0:64, h, csl], in0=qrel[rs, :],
                                in1=sbt[rs, csl], op=AluOpType.mult)
                            nc.vector.tensor_tensor(
                                out=qq[64:128, h, csl], in0=qrel[rs, :],
                                in1=cbt[rs, csl], op=AluOpType.mult)

            # ---- Phases E/F/G interleaved ----
            with (
                tc.tile_pool(name='ef', bufs=3) as efp,
                tc.tile_pool(name='psE', bufs=4, space='PSUM') as pse,
                tc.tile_pool(name='go', bufs=4) as gop,
                tc.tile_pool(name='psG', bufs=4, space='PSUM') as psg,
            ):
                def emit_attn_ln2(i):
                    rsl = slice(i * 128, (i + 1) * 128)
                    yt = efp.tile([128, H, 64], BF, tag='yt')
                    z16 = efp.tile([128, H], FP32, tag='z16')
                    pas = []
                    for g in range(4):
                        pa = pse.tile([128, 4 * 65], FP32, tag='psE',
                                      name=f'pa_{i}_{g}')
                        pas.append(pa)
                        for hh in range(4):
                            h = 4 * g + hh
                            nc.tensor.matmul(pa[:, hh * 65:(hh + 1) * 65],
                                             lhsT=qq[:, h, rsl],
                                             rhs=kvb[:, h * 65:(h + 1) * 65],
                                             start=True, stop=True)
                        pav = pa[:].rearrange('p (h c) -> p h c', c=65)
                        nc.vector.tensor_scalar(
                            out=z16[:, g * 4:(g + 1) * 4], in0=pav[:, :, 64],
                            scalar1=EPS_ATTN, scalar2=None, op0=AluOpType.max)
                    nc.vector.reciprocal(out=z16, in_=z16)
                    for g in range(4):
                        pav = pas[g][:].rearrange('p (h c) -> p h c', c=65)
                        zb = z16[:, g * 4:(g + 1) * 4].broadcast_to((128, 4, 64))
                        nc.vector.tensor_tensor(out=yt[:, g * 4:(g + 1) * 4, :],
                                                in0=pav[:, :, 0:64], in1=zb,
                                                op=AluOpType.mult)
                    ytf = yt[:].rearrange('p h d -> p (h d)')
                    # residual + LN2 stats via accumulating side-outputs
                    s1 = efp.tile([128, 1], FP32, tag='s1')
                    s2 = efp.tile([128, 1], FP32, tag='s2')
                    ysq = efp.tile([128, E], BF, tag='ysq')
                    nc.vector.scalar_tensor_tensor(
                        out=ytf, in0=ytf, scalar=1.0, in1=qn[:, i, :],
                        op0=AluOpType.mult, op1=AluOpType.add, accum_out=s1)
                    nc.vector.scalar_tensor_tensor(
                        out=ysq, in0=ytf, scalar=1.0, in1=ytf,
                        op0=AluOpType.mult, op1=AluOpType.mult, accum_out=s2)
                    mu2c = efp.tile([128, 1], FP32, tag='mu2c')
                    varc = efp.tile([128, 1], FP32, tag='varc')
                    nc.vector.tensor_scalar(out=mu2c, in0=s1, scalar1=1.0 / E,
                                            scalar2=None, op0=AluOpType.mult)
                    nc.vector.tensor_scalar(out=varc, in0=s2, scalar1=1.0 / E,
                                            scalar2=None, op0=AluOpType.mult)
                    m2s = efp.tile([128, 1], FP32, tag='m2s')
                    nc.vector.tensor_scalar(out=m2s, in0=mu2c, scalar1=mu2c,
                                            scalar2=None, op0=AluOpType.mult)
                    nc.vector.tensor_tensor(out=varc, in0=varc, in1=m2s,
                                            op=AluOpType.subtract)
                    rstd2 = efp.tile([128, 1], FP32, tag='rstd2')
                    nc.scalar.activation(out=rstd2, in_=varc, func=AF.Sqrt,
                                         bias=eps1, scale=1.0)
                    nc.vector.reciprocal(out=rstd2, in_=rstd2)
                    nb2 = efp.tile([128, 1], FP32, tag='nb2')
                    nc.vector.tensor_scalar(out=nb2, in0=mu2c,
                                            scalar1=rstd2, scalar2=-1.0,
                                            op0=AluOpType.mult,
                                            op1=AluOpType.mult)
                    nc.scalar.activation(out=xh[:, i, :], in_=ytf, func=AF.Identity,
                                         bias=nb2, scale=rstd2)
                    nc.gpsimd.dma_start(out=xh_dram[rsl, :], in_=xh[:, i, :])

                def emit_xh_transpose(tsl, jlo, jhi):
                    for j in range(jlo, jhi):
                        qeng = nc.sync if j % 2 else nc.scalar
                        qeng.dma_start(out=xhT[:, j, tsl],
                                       in_=xh_dram[tsl, j * 128:(j + 1) * 128],
                                       transpose=True)
                    for j in range(jlo, jhi):
                        nc.vector.tensor_copy(
                            out=xhT8[:, j // 2, j % 2, tsl],
                            in_=xhT[:, j, tsl])

                def emit_o(i):
                    isl = slice(i * 128, (i + 1) * 128)
                    for ch in range(2):
                        csl = slice(ch * 512, (ch + 1) * 512)
                        po = psg.tile([128, 512], FP32, tag='psG',
                                      name=f'po_{i}_{ch}')
                        for k2 in range(NK2):
                            nc.tensor.matmul(
                                po, lhsT=xhT8[:, k2, :, isl],
                                rhs=wo_t[:, k2, :, csl],
                                perf_mode=DR,
                                start=(k2 == 0), stop=(k2 == NK2 - 1))
                        ot = gop.tile([128, 512], BF, tag='ot')
                        xres = xh[:, i, csl]
                        if has_g2:
                            xg2 = gop.tile([128, 512], FP32, tag='xg2')
                            nc.vector.tensor_tensor(out=xg2, in0=xh[:, i, csl],
                                                    in1=g2b[:, csl],
                                                    op=AluOpType.mult)
                            xres = xg2
                        if ch == 0:
                            nc.vector.scalar_tensor_tensor(
                                out=ot, in0=po, scalar=IWS, in1=xres,
                                op0=AluOpType.mult, op1=AluOpType.add)
                        else:
                            # off the vector engine: scale on scalar, add on pool
                            ot1 = gop.tile([128, 512], BF, tag='ot1')
                            nc.scalar.activation(out=ot1, in_=po, func=AF.Identity,
                                                 scale=IWS)
                            nc.gpsimd.tensor_tensor(out=ot, in0=ot1, in1=xres,
                                                    op=AluOpType.add)
                        if has_b2o:
                            nc.vector.tensor_tensor(out=ot, in0=ot,
                                                    in1=b2ob[:, csl],
                                                    op=AluOpType.add)
                        qeng = nc.scalar if ch else nc.sync
                        qeng.dma_start(out=d_out[isl, csl], in_=ot)

                emit_attn_ln2(0)
                emit_attn_ln2(1)
                emit_attn_ln2(2)
                emit_attn_ln2(3)
                emit_xh_transpose(slice(0, 512), 0, NJ)
                emit_attn_ln2(4)
                emit_o(0)
                emit_attn_ln2(5)
                emit_o(1)
                emit_attn_ln2(6)
                emit_o(2)
                emit_attn_ln2(7)
                emit_o(3)
                emit_xh_transpose(slice(512, 1024), 0, NJ)
                emit_o(4)
                emit_o(5)
                emit_o(6)
                emit_o(7)

    nc.compile()
    return nc


def _get_program(flags):
    if flags not in _BUILD_CACHE:
        _BUILD_CACHE[flags] = _build_program(flags)
    return _BUILD_CACHE[flags]


def _phm_weight(A, S):
    f = A.shape[0]
    din, dout = f * S.shape[1], f * S.shape[2]
    W = np.einsum('nij,nkl->ikjl', np.asarray(A, np.float32), np.asarray(S, np.float32))
    return np.ascontiguousarray(W.reshape(din, dout))


def _w8(W):
    """[E, E] fp32 -> [128, NK2, 2, E] fp8 with x64 scale."""
    Wv = (W * WS).reshape(NK2, 2, 128, E)
    return np.ascontiguousarray(np.transpose(Wv, (2, 0, 1, 3))).astype(E4M3)


def kernel(**inputs):
    query = np.asarray(inputs['query'], np.float32)
    g1 = np.asarray(inputs['g1'], np.float32)
    b1 = np.asarray(inputs['b1'], np.float32)
    g2 = np.asarray(inputs['g2'], np.float32)
    b2 = np.asarray(inputs['b2'], np.float32)
    qb = np.asarray(inputs['qb'], np.float32)
    kb = np.asarray(inputs['kb'], np.float32)
    vb = np.asarray(inputs['vb'], np.float32)
    ob = np.asarray(inputs['ob'], np.float32)

    Wq = _phm_weight(inputs['qA'], inputs['qS'])
    Wk = _phm_weight(inputs['kA'], inputs['kS'])
    Wv = _phm_weight(inputs['vA'], inputs['vS'])
    Wo = _phm_weight(inputs['oA'], inputs['oS'])

    has_g1b1 = not (np.all(g1 == 1.0) and np.all(b1 == 0.0))
    has_qb = bool(np.any(qb != 0.0))
    has_kb = bool(np.any(kb != 0.0))
    has_vb = bool(np.any(vb != 0.0))
    has_g2 = not np.all(g2 == 1.0)
    # final = xh@(g2*Wo) + xh*g2 + C,  C = b2@Wo + ob + b2
    C = b2 @ Wo + ob + b2
    has_b2o = bool(np.any(C != 0.0))
    general_q = has_g1b1 or has_qb
    flags = (has_g1b1, has_qb, has_kb, has_vb, has_g2, has_b2o)

    nc = _get_program(flags)

    Wg = g2[:, None] * Wo
    Wq_eff = g1[:, None] * Wq if has_g1b1 else Wq
    wq_b = _w8(Wq_eff)
    wk_b = _w8(Wk)
    wv_b = _w8(Wv)
    wo_b = _w8(Wg)

    s_full = np.sin((np.pi / 2) * np.arange(1, L + 1, dtype=np.float32) / L)
    c_full = np.cos((np.pi / 2) * np.arange(1, L + 1, dtype=np.float32) / L)

    in_maps = []
    for core in range(NCORES):
        b = core // 2
        l0 = (core % 2) * T
        x = np.ascontiguousarray(query[l0:l0 + T, b, :])
        xT = np.transpose(x).reshape(NK2, 2, 128, T)
        s = s_full[l0:l0 + T]
        c = c_full[l0:l0 + T]
        im = {
            'x_tm': x.astype(BF16),
            'x_fm8': np.ascontiguousarray(np.transpose(xT, (2, 0, 1, 3))).astype(E4M3),
            'wq8': wq_b, 'wk8': wk_b, 'wv8': wv_b, 'wo8': wo_b,
            's_bcast': np.ascontiguousarray(
                np.broadcast_to(s / WS, (128, T))).astype(BF16),
            'c_bcast': np.ascontiguousarray(
                np.broadcast_to(c / WS, (128, T))).astype(BF16),
            's_cols': np.ascontiguousarray((s / 8.0).reshape(NT, 128).T),
            'c_cols': np.ascontiguousarray((c / 8.0).reshape(NT, 128).T),
        }
        if has_g1b1:
            im['g1_b'] = np.ascontiguousarray(np.broadcast_to(g1, (128, E)))
            im['b1_b'] = np.ascontiguousarray(np.broadcast_to(b1, (128, E)))
        if general_q:
            cj = ((b1 @ Wq if has_g1b1 else np.zeros(E, np.float32)) + qb) * WS
            im['q_cj'] = np.ascontiguousarray(cj.reshape(NJ, 128).T)
        if has_kb:
            im['kb_b'] = np.ascontiguousarray(np.broadcast_to(kb * WS, (128, E)))
        if has_vb:
            im['vb_b'] = np.ascontiguousarray(np.broadcast_to(vb * WS, (128, E)))
        if has_g2:
            im['g2_b'] = np.ascontiguousarray(np.broadcast_to(g2, (128, E)))
        if has_b2o:
            im['b2o_b'] = np.ascontiguousarray(np.broadcast_to(C, (128, E)))
        in_maps.append(im)

    trace = bool(os.environ.get('KERNEL_TRACE'))
    res = run_bass_kernel_spmd(nc, in_maps, list(range(NCORES)), trace=trace)
    kernel._last_exec_ns = res.exec_time_ns

    out = np.empty((L, N, E), np.float32)
    for core in range(NCORES):
        b = core // 2
        l0 = (core % 2) * T
        out[l0:l0 + T, b, :] = res.results[core]['out'].astype(np.float32)
    return out


kernel._last_exec_ns = None


# revision 22
# speedup vs baseline: 1.6716x; 1.0334x over previous
"""Cosformer attention Bass kernel for 8 trn2 NeuronCores — fp8 edition.

Sharding: core c handles batch c//2, sequence half c%2 (1024 tokens).
Per-head linear-attention state (kv, ksum) is AllReduce'd (bf16) between
the two cores sharing a batch.

The 4 big projection matmuls run in fp8e4m3 with DoubleRow perf mode
(K=256 per instruction, ~2x bf16 rate). Weights are scaled x64 on host to
escape the fp8 subnormal zone; the scales cancel through the cosformer
z-normalization or are folded into epilogues.

LN1 folding: rstd cancels through z-normalization (relu commutes with
positive per-token scale), so the Q projection consumes centered x only:
x_c = x - mu, built feature-major from a partition-broadcast of mu. This
removes the qn transposes entirely; qn stays SBUF-resident for the
residual.

The final residual (+ out) is applied as an epilogue add (NOT folded into
Wo: the identity diagonal would quantize at ~6% in fp8).
"""

import sys

for _p in ('/opt/trn_rl_repo',):
    if _p not in sys.path:
        sys.path.insert(0, _p)

import importlib.util as _ilu
import os

os.environ.setdefault('NEURON_RT_RESET_CORES', '1')

# The image's antenv lacks axon_hooks (needed for trace=True); register ours.
if 'antenv.axon_hooks' not in sys.modules:
    _hp = '/opt/trn_rl_repo/antenv/axon_hooks.py'
    if os.path.exists(_hp):
        _spec = _ilu.spec_from_file_location('antenv.axon_hooks', _hp)
        _mod = _ilu.module_from_spec(_spec)
        _spec.loader.exec_module(_mod)
        sys.modules['antenv.axon_hooks'] = _mod
    else:
        import types as _types

        _mod = _types.ModuleType('antenv.axon_hooks')
        _mod._hook = None
        _mod.set_axon_ntff_profile_hook = lambda h: setattr(_mod, '_hook', h)
        _mod.get_axon_ntff_profile_hook = lambda: _mod._hook
        sys.modules['antenv.axon_hooks'] = _mod


def _register_ntff_hook():
    """If boot didn't register the NTFF profile hook (image antenv lacks
    axon_hooks), drive NRT profiling via ctypes into libaxon_pjrt.so."""
    import contextlib
    import ctypes

    mod = sys.modules['antenv.axon_hooks']
    if mod.get_axon_ntff_profile_hook() is not None:
        return
    so_path = '/opt/axon/libaxon_pjrt.so'
    if not os.path.exists(so_path):
        return
    try:
        lib = ctypes.CDLL(so_path)
        if not hasattr(lib, 'axon_start_nrt_profile'):
            return
        lib.axon_start_nrt_profile.argtypes = [
            ctypes.POINTER(ctypes.c_int64), ctypes.c_size_t]
        lib.axon_start_nrt_profile.restype = ctypes.c_int64
        lib.axon_stop_nrt_profile.argtypes = [ctypes.c_char_p]
        lib.axon_stop_nrt_profile.restype = ctypes.c_int64
    except OSError:
        return

    @contextlib.contextmanager
    def _hook(output_dir, device_ids):
        import jax
        jax.devices()
        if device_ids:
            ids = (ctypes.c_int64 * len(device_ids))(*device_ids)
            rc = lib.axon_start_nrt_profile(ids, len(device_ids))
        else:
            rc = lib.axon_start_nrt_profile(None, 0)
        if rc != 0:
            raise RuntimeError(f'axon_start_nrt_profile rc={rc}')
        try:
            yield
        finally:
            n = lib.axon_stop_nrt_profile(str(output_dir).encode())
            if n < 0:
                raise RuntimeError(f'axon_stop_nrt_profile rc={n}')

    mod.set_axon_ntff_profile_hook(_hook)


_register_ntff_hook()

import numpy as np
import ml_dtypes

import concourse.bass as bass
import concourse.tile as tile
from concourse import bacc, mybir
from concourse.alu_op_type import AluOpType
from concourse.bass_utils import run_bass_kernel_spmd

BF16 = ml_dtypes.bfloat16
E4M3 = ml_dtypes.float8_e4m3
FP32 = mybir.dt.float32
BF = mybir.dt.bfloat16
F8 = mybir.dt.float8e4
AF = mybir.ActivationFunctionType
DR = mybir.MatmulPerfMode.DoubleRow

L, N, E, H, D = 2048, 4, 1024, 16, 64
T = 1024            # tokens per core
NT = T // 128       # 8 token tiles
NK2 = 4             # DoubleRow contraction steps (256 features each)
NJ = E // 128       # 8 output-feature tiles
NCORES = 8
EPS_LN = 1e-5
EPS_ATTN = 1e-6
WS = 64.0           # host weight scale
IWS = 1.0 / WS

_BUILD_CACHE = {}


def _build_program(flags):
    """flags: (has_g1b1, has_qb, has_kb, has_vb, has_g2, has_b2o)."""
    has_g1b1, has_qb, has_kb, has_vb, has_g2, has_b2o = flags
    general_q = has_g1b1 or has_qb   # r no longer cancels for the Q path

    nc = bacc.Bacc("TRN2", target_bir_lowering=False, debug=False,
                   num_devices=NCORES)

    # ---- DRAM I/O ----
    d_x_tm = nc.dram_tensor('x_tm', [T, E], BF, kind='ExternalInput')
    d_x_fm8 = nc.dram_tensor('x_fm8', [128, NK2, 2, T], F8, kind='ExternalInput')
    d_wq = nc.dram_tensor('wq8', [128, NK2, 2, E], F8, kind='ExternalInput')
    d_wk = nc.dram_tensor('wk8', [128, NK2, 2, E], F8, kind='ExternalInput')
    d_wv = nc.dram_tensor('wv8', [128, NK2, 2, E], F8, kind='ExternalInput')
    d_wo = nc.dram_tensor('wo8', [128, NK2, 2, E], F8, kind='ExternalInput')
    d_sb = nc.dram_tensor('s_bcast', [128, T], BF, kind='ExternalInput')
    d_cb = nc.dram_tensor('c_bcast', [128, T], BF, kind='ExternalInput')
    d_scol = nc.dram_tensor('s_cols', [128, NT], FP32, kind='ExternalInput')
    d_ccol = nc.dram_tensor('c_cols', [128, NT], FP32, kind='ExternalInput')
    d_g1b = nc.dram_tensor('g1_b', [128, E], FP32, kind='ExternalInput') if has_g1b1 else None
    d_b1b = nc.dram_tensor('b1_b', [128, E], FP32, kind='ExternalInput') if has_g1b1 else None
    d_qcj = nc.dram_tensor('q_cj', [128, NJ], FP32, kind='ExternalInput') if general_q else None
    d_kbb = nc.dram_tensor('kb_b', [128, E], FP32, kind='ExternalInput') if has_kb else None
    d_vbb = nc.dram_tensor('vb_b', [128, E], FP32, kind='ExternalInput') if has_vb else None
    d_g2b = nc.dram_tensor('g2_b', [128, E], FP32, kind='ExternalInput') if has_g2 else None
    d_b2ob = nc.dram_tensor('b2o_b', [128, E], FP32, kind='ExternalInput') if has_b2o else None
    d_out = nc.dram_tensor('out', [T, E], BF, kind='ExternalOutput')

    RG = [[0, 1], [2, 3], [4, 5], [6, 7]]

    with tile.TileContext(nc) as tc:
        with (
            tc.tile_pool(name='persist', bufs=1) as pp,
            tc.tile_pool(name='wpool', bufs=2) as wp,
            tc.tile_pool(name='dram', bufs=1, space='DRAM') as dp,
        ):
            # ---- persistent tiles ----
            sbt = pp.tile([128, T], BF, tag='sbt')          # s/WS bcast
            cbt = pp.tile([128, T], BF, tag='cbt')
            scol = pp.tile([128, NT], FP32, tag='scol')     # s/8 cols
            ccol = pp.tile([128, NT], FP32, tag='ccol')
            eps1 = pp.tile([128, 1], FP32, tag='eps1')
            qn = pp.tile([128, NT, E], BF, tag='qn')        # LN1 out, token-major
            xh = pp.tile([128, NT, E], BF, tag='xh')        # LN2 out, token-major
            xc8 = pp.tile([128, NK2, 2, T], F8, tag='xc8')  # centered x, fm
            ksc = pp.tile([128, NT, H, 128], F8, tag='ksc') # 8*k*[s|c], tok-major
            v_aug = pp.tile([128, NT, H, 65], F8, tag='vaug')
            qq = pp.tile([128, H, T], BF, tag='qq')         # q~*[s|c] per head, fm
            kvb = pp.tile([128, H * 65], BF, tag='kvb')     # reduced kv (true scale)
            kvp = pp.tile([128, H * 65], BF, tag='kvp')
            xhT = pp.tile([128, NJ, T], BF, tag='xhT')      # transposed xh
            xhT8 = pp.tile([128, NK2, 2, T], F8, tag='xhT8')
            negmu_cols = pp.tile([128, NT], BF, tag='nmu')  # -mu per token
            mu_row = pp.tile([1, T], BF, tag='murow')
            mu_bc = pp.tile([128, T], BF, tag='mubc')
            rst_cols = pp.tile([128, NT], FP32, tag='rst') if general_q else None
            r_row = pp.tile([1, T], FP32, tag='rrow') if general_q else None
            r_bc = pp.tile([128, T], FP32, tag='rbc') if general_q else None

            g1b = b1b = qcj = kbb = vbb = g2b = b2ob = None
            if has_g1b1:
                g1b = pp.tile([128, E], FP32, tag='g1b')
                b1b = pp.tile([128, E], FP32, tag='b1b')
                nc.gpsimd.dma_start(out=g1b, in_=d_g1b[:])
                nc.gpsimd.dma_start(out=b1b, in_=d_b1b[:])
            if general_q:
                qcj = pp.tile([128, NJ], FP32, tag='qcj')
                nc.gpsimd.dma_start(out=qcj, in_=d_qcj[:])
            if has_kb:
                kbb = pp.tile([128, E], FP32, tag='kbb')
                nc.gpsimd.dma_start(out=kbb, in_=d_kbb[:])
            if has_vb:
                vbb = pp.tile([128, E], FP32, tag='vbb')
                nc.gpsimd.dma_start(out=vbb, in_=d_vbb[:])
            if has_g2:
                g2b = pp.tile([128, E], FP32, tag='g2b')
                nc.gpsimd.dma_start(out=g2b, in_=d_g2b[:])
            if has_b2o:
                b2ob = pp.tile([128, E], FP32, tag='b2ob')
                nc.gpsimd.dma_start(out=b2ob, in_=d_b2ob[:])

            # DRAM scratch
            xh_dram = dp.tile([T, E], BF)
            kv_cc_in = dp.tile([128, H * 65], BF)
            kv_cc_out = dp.tile([128, H * 65], BF)

            # ---- front-loaded DMAs ----
            # scalar queue: x_tm tiles (LN1 needs them first) interleaved
            # with Wk chunks (PE needs chunk 0 at ~1.5us)
            xfm = pp.tile([128, NK2, 2, T], F8, tag='xfm')
            wk_t = wp.tile([128, NK2, 2, E], F8, tag='W')
            with tc.tile_pool(name='xtiles', bufs=8) as xp:
                xts = []
                for i in range(NT):
                    xt = xp.tile([128, E], BF, tag='xt', name=f'xt{i}')
                    xts.append(xt)
                nc.scalar.dma_start(out=xts[0], in_=d_x_tm[0:128, :])
                nc.scalar.dma_start(out=wk_t[:, 0], in_=d_wk[:, 0])
                nc.scalar.dma_start(out=xts[1], in_=d_x_tm[128:256, :])
                nc.scalar.dma_start(out=wk_t[:, 1], in_=d_wk[:, 1])
                for k2 in range(NK2):
                    nc.sync.dma_start(out=xfm[:, k2], in_=d_x_fm8[:, k2])
                nc.scalar.dma_start(out=wk_t[:, 2], in_=d_wk[:, 2])
                nc.scalar.dma_start(out=wk_t[:, 3], in_=d_wk[:, 3])
                for i in range(2, NT):
                    nc.scalar.dma_start(out=xts[i],
                                        in_=d_x_tm[i * 128:(i + 1) * 128, :])
                nc.sync.dma_start(out=scol, in_=d_scol[:])
                nc.sync.dma_start(out=ccol, in_=d_ccol[:])
                nc.sync.dma_start(out=sbt, in_=d_sb[:])
                nc.sync.dma_start(out=cbt, in_=d_cb[:])
                nc.vector.memset(eps1, EPS_LN)
                nc.vector.memset(v_aug[:, :, :, 64:65], 8.0)
                wv_t = wp.tile([128, NK2, 2, E], F8, tag='W')
                for k2 in range(NK2):
                    nc.gpsimd.dma_start(out=wv_t[:, k2], in_=d_wv[:, k2])

                # ---- Phase A: LN1 per token tile (vector stats, scalar app) ----
                with tc.tile_pool(name='ln1', bufs=4) as ap:
                    for i in range(NT):
                        xt = xts[i]
                        st = ap.tile([128, 2, 6], FP32, tag='st')
                        xg = xt[:].rearrange('p (g d) -> p g d', g=2)
                        nc.vector.bn_stats(out=st[:, 0, :], in_=xg[:, 0, :])
                        nc.vector.bn_stats(out=st[:, 1, :], in_=xg[:, 1, :])
                        mv = ap.tile([128, 2], FP32, tag='mv')
                        nc.vector.bn_aggr(out=mv, in_=st)
                        rstd = ap.tile([128, 1], FP32, tag='rstd')
                        nc.scalar.activation(out=rstd, in_=mv[:, 1:2], func=AF.Sqrt,
                                             bias=eps1, scale=1.0)
                        nc.vector.reciprocal(out=rstd, in_=rstd)
                        # -mu (bf16) for the x_c broadcast path
                        nc.vector.tensor_scalar(out=negmu_cols[:, i:i + 1],
                                                in0=mv[:, 0:1], scalar1=-1.0,
                                                scalar2=None, op0=AluOpType.mult)
                        if general_q:
                            nc.vector.tensor_copy(out=rst_cols[:, i:i + 1],
                                                  in_=rstd)
                        # qn = (x - mu) * rstd (+affine) via scalar ACT
                        nbias = ap.tile([128, 1], FP32, tag='nb')
                        nc.vector.tensor_scalar(out=nbias, in0=mv[:, 0:1],
                                                scalar1=rstd, scalar2=-1.0,
                                                op0=AluOpType.mult,
                                                op1=AluOpType.mult)
                        if has_g1b1:
                            qtmp = ap.tile([128, E], FP32, tag='qtmp')
                            nc.scalar.activation(out=qtmp, in_=xt,
                                                 func=AF.Identity,
                                                 bias=nbias, scale=rstd)
                            nc.vector.scalar_tensor_tensor(
                                out=qn[:, i, :], in0=qtmp, scalar=1.0, in1=g1b,
                                op0=AluOpType.mult, op1=AluOpType.mult)
                            nc.vector.tensor_tensor(out=qn[:, i, :],
                                                    in0=qn[:, i, :], in1=b1b,
                                                    op=AluOpType.add)
                        else:
                            nc.scalar.activation(out=qn[:, i, :], in_=xt,
                                                 func=AF.Identity, bias=nbias,
                                                 scale=rstd)
                        # gather -mu into a flat [1, T] row (cross-partition DMA)
                        nc.sync.dma_start(
                            out=mu_row[:, i * 128:(i + 1) * 128],
                            in_=negmu_cols[:, i:i + 1])
                        if general_q:
                            nc.sync.dma_start(
                                out=r_row[:, i * 128:(i + 1) * 128],
                                in_=rst_cols[:, i:i + 1])

                # mu flat row -> partition broadcast -> centered x (fp8)
                nc.gpsimd.partition_broadcast(mu_bc, mu_row, channels=128)
                for k2 in range(NK2):
                    for h in range(2):
                        nc.vector.tensor_tensor(out=xc8[:, k2, h, :],
                                                in0=xfm[:, k2, h, :],
                                                in1=mu_bc, op=AluOpType.add)
                if general_q:
                    nc.gpsimd.partition_broadcast(r_bc, r_row, channels=128)

                # ---- Phases B1/B2: K and V projections (fp8 DR) ----
                with tc.tile_pool(name='psB', bufs=8, space='PSUM') as psb:
                    def phm_tok_major(w_t, epilogue, nm):
                        for half in range(2):
                            ptiles = {}
                            for i in range(4 * half, 4 * half + 4):
                                for ch in range(2):
                                    pt = psb.tile([128, 512], FP32, tag='psB',
                                                  name=f'pb{nm}_{i}_{ch}')
                                    ptiles[i, ch] = pt
                            for k2 in range(NK2):
                                for i in range(4 * half, 4 * half + 4):
                                    isl = slice(i * 128, (i + 1) * 128)
                                    for ch in range(2):
                                        csl = slice(ch * 512, (ch + 1) * 512)
                                        nc.tensor.matmul(
                                            ptiles[i, ch],
                                            lhsT=xfm[:, k2, :, isl],
                                            rhs=w_t[:, k2, :, csl],
                                            perf_mode=DR,
                                            start=(k2 == 0), stop=(k2 == NK2 - 1))
                            for i in range(4 * half, 4 * half + 4):
                                for ch in range(2):
                                    epilogue(i, ch, ptiles[i, ch])

                    def k_epilogue(i, ch, pk):
                        csl = slice(ch * 512, (ch + 1) * 512)
                        if has_kb:
                            nc.vector.tensor_tensor(out=pk, in0=pk,
                                                    in1=kbb[:, csl],
                                                    op=AluOpType.add)
                        pkv = pk[:].rearrange('p (h d) -> p h d', d=64)
                        hsl = slice(ch * 8, (ch + 1) * 8)
                        # sin half on scalar engine, cos half on vector
                        nc.scalar.activation(
                            out=ksc[:, i, hsl, 0:64],
                            in_=pk[:].rearrange('p (h d) -> p h d', d=64),
                            func=AF.Relu, scale=scol[:, i:i + 1])
                        nc.vector.tensor_scalar(
                            out=ksc[:, i, hsl, 64:128], in0=pkv,
                            scalar1=0.0, scalar2=ccol[:, i:i + 1],
                            op0=AluOpType.max, op1=AluOpType.mult)

                    def v_epilogue(i, ch, pv):
                        csl = slice(ch * 512, (ch + 1) * 512)
                        if has_vb:
                            nc.vector.tensor_tensor(out=pv, in0=pv,
                                                    in1=vbb[:, csl],
                                                    op=AluOpType.add)
                        hsl = slice(ch * 8, (ch + 1) * 8)
                        nc.vector.tensor_scalar(
                            out=v_aug[:, i, hsl, 0:64],
                            in0=pv[:].rearrange('p (h d) -> p h d', d=64),
                            scalar1=0.125, scalar2=None, op0=AluOpType.mult)

                    phm_tok_major(wk_t, k_epilogue, 'k')
                    phm_tok_major(wv_t, v_epilogue, 'v')

                # Wq load (wk slot frees): gpsimd queue (scalar is busy with
                # epilogue ACTs; gpsimd is idle here)
                wq_t = wp.tile([128, NK2, 2, E], F8, tag='W')
                for k2 in range(NK2):
                    nc.gpsimd.dma_start(out=wq_t[:, k2], in_=d_wq[:, k2])

                # ---- Phase C: per-head kv partials (fp8 DR) + AllReduce ----
                with tc.tile_pool(name='psC', bufs=8, space='PSUM') as psc:
                    for h in range(H):
                        pkv = psc.tile([128, 65], FP32, tag='psC', name=f'kv{h}')
                        for i2 in range(NT // 2):
                            nc.tensor.matmul(
                                pkv,
                                lhsT=ksc[:, 2 * i2:2 * i2 + 2, h, :],
                                rhs=v_aug[:, 2 * i2:2 * i2 + 2, h, :],
                                perf_mode=DR,
                                start=(i2 == 0), stop=(i2 == NT // 2 - 1))
                        nc.vector.tensor_scalar(
                            out=kvp[:, h * 65:(h + 1) * 65], in0=pkv,
                            scalar1=IWS, scalar2=None, op0=AluOpType.mult)

                nc.gpsimd.dma_start(out=kv_cc_in[:], in_=kvp)
                nc.gpsimd.collective_compute(
                    'AllReduce', AluOpType.add,
                    ins=[kv_cc_in.opt()], outs=[kv_cc_out.opt()],
                    replica_groups=RG)

            # Wo load (wv slot frees): gpsimd queue, during the collective
            wo_t = wp.tile([128, NK2, 2, E], F8, tag='W')
            for k2 in range(NK2):
                nc.gpsimd.dma_start(out=wo_t[:, k2], in_=d_wo[:, k2])
            nc.gpsimd.dma_start(out=kvb, in_=kv_cc_out[:])

            # ---- Phase D: Q projection (fp8 DR from x_c) ----
            with (
                tc.tile_pool(name='psD', bufs=4, space='PSUM') as psd,
                tc.tile_pool(name='qsb', bufs=4) as qsb,
            ):
                for ch in range(2):
                    csl = slice(ch * 512, (ch + 1) * 512)
                    for j in range(NJ):
                        jsl = slice(j * 128, (j + 1) * 128)
                        pq = psd.tile([128, 512], FP32, tag='psD',
                                      name=f'pq{ch}_{j}')
                        for k2 in range(NK2):
                            nc.tensor.matmul(
                                pq, lhsT=wq_t[:, k2, :, jsl],
                                rhs=xc8[:, k2, :, csl],
                                perf_mode=DR,
                                start=(k2 == 0), stop=(k2 == NK2 - 1))
                        if general_q:
                            nc.vector.tensor_tensor(out=pq, in0=pq,
                                                    in1=r_bc[:, csl],
                                                    op=AluOpType.mult)
                        qrel = qsb.tile([128, 512], BF, tag='qrel')
                        if general_q:
                            nc.scalar.activation(out=qrel, in_=pq, func=AF.Relu,
                                                 bias=qcj[:, j:j + 1])
                        else:
                            nc.scalar.activation(out=qrel, in_=pq, func=AF.Relu)
                        for hh in range(2):
                            h = 2 * j + hh
                            rs = slice(hh * 64, (hh + 1) * 64)
                            nc.vector.tensor_tensor(
                                out=qq[0:64, h, csl], in0=qrel[rs, :],
                                in1=sbt[rs, csl], op=AluOpType.mult)
                            nc.vector.tensor_tensor(
                                out=qq[64:128, h, csl], in0=qrel[rs, :],
                                in1=cbt[rs, csl], op=AluOpType.mult)

            # ---- Phases E/F/G interleaved ----
            with (
                tc.tile_pool(name='ef', bufs=3) as efp,
                tc.tile_pool(name='psE', bufs=4, space='PSUM') as pse,
                tc.tile_pool(name='go', bufs=4) as gop,
                tc.tile_pool(name='psG', bufs=4, space='PSUM') as psg,
            ):
                def emit_attn_ln2(i):
                    rsl = slice(i * 128, (i + 1) * 128)
                    yt = efp.tile([128, H, 64], BF, tag='yt')
                    z16 = efp.tile([128, H], FP32, tag='z16')
                    pas = []
                    for g in range(4):
                        pa = pse.tile([128, 4 * 65], FP32, tag='psE',
                                      name=f'pa_{i}_{g}')
                        pas.append(pa)
                        for hh in range(4):
                            h = 4 * g + hh
                            nc.tensor.matmul(pa[:, hh * 65:(hh + 1) * 65],
                                             lhsT=qq[:, h, rsl],
                                             rhs=kvb[:, h * 65:(h + 1) * 65],
                                             start=True, stop=True)
                        pav = pa[:].rearrange('p (h c) -> p h c', c=65)
                        nc.vector.tensor_scalar(
                            out=z16[:, g * 4:(g + 1) * 4], in0=pav[:, :, 64],
                            scalar1=EPS_ATTN, scalar2=None, op0=AluOpType.max)
                    nc.vector.reciprocal(out=z16, in_=z16)
                    for g in range(4):
                        pav = pas[g][:].rearrange('p (h c) -> p h c', c=65)
                        zb = z16[:, g * 4:(g + 1) * 4].broadcast_to((128, 4, 64))
                        nc.vector.tensor_tensor(out=yt[:, g * 4:(g + 1) * 4, :],
                                                in0=pav[:, :, 0:64], in1=zb,
                                                op=AluOpType.mult)
                    ytf = yt[:].rearrange('p h d -> p (h d)')
                    nc.vector.tensor_tensor(out=ytf, in0=ytf, in1=qn[:, i, :],
                                            op=AluOpType.add)
                    # LN2 stats
                    st2 = efp.tile([128, 2, 6], FP32, tag='st2')
                    yg = yt[:].rearrange('p (g x) d -> p g (x d)', g=2)
                    nc.vector.bn_stats(out=st2[:, 0, :], in_=yg[:, 0, :])
                    nc.vector.bn_stats(out=st2[:, 1, :], in_=yg[:, 1, :])
                    mv2 = efp.tile([128, 2], FP32, tag='mv2')
                    nc.vector.bn_aggr(out=mv2, in_=st2)
                    mu2c = mv2[:, 0:1]
                    rstd2 = efp.tile([128, 1], FP32, tag='rstd2')
                    nc.scalar.activation(out=rstd2, in_=mv2[:, 1:2], func=AF.Sqrt,
                                         bias=eps1, scale=1.0)
                    nc.vector.reciprocal(out=rstd2, in_=rstd2)
                    nb2 = efp.tile([128, 1], FP32, tag='nb2')
                    nc.vector.tensor_scalar(out=nb2, in0=mu2c,
                                            scalar1=rstd2, scalar2=-1.0,
                                            op0=AluOpType.mult,
                                            op1=AluOpType.mult)
                    nc.scalar.activation(out=xh[:, i, :], in_=ytf, func=AF.Identity,
                                         bias=nb2, scale=rstd2)
                    nc.gpsimd.dma_start(out=xh_dram[rsl, :], in_=xh[:, i, :])

                def emit_xh_transpose(tsl, jlo, jhi):
                    for j in range(jlo, jhi):
                        qeng = nc.sync if j % 2 else nc.scalar
                        qeng.dma_start(out=xhT[:, j, tsl],
                                       in_=xh_dram[tsl, j * 128:(j + 1) * 128],
                                       transpose=True)
                    for j in range(jlo, jhi):
                        nc.vector.tensor_copy(
                            out=xhT8[:, j // 2, j % 2, tsl],
                            in_=xhT[:, j, tsl])

                def emit_o(i):
                    isl = slice(i * 128, (i + 1) * 128)
                    for ch in range(2):
                        csl = slice(ch * 512, (ch + 1) * 512)
                        po = psg.tile([128, 512], FP32, tag='psG',
                                      name=f'po_{i}_{ch}')
                        for k2 in range(NK2):
                            nc.tensor.matmul(
                                po, lhsT=xhT8[:, k2, :, isl],
                                rhs=wo_t[:, k2, :, csl],
                                perf_mode=DR,
                                start=(k2 == 0), stop=(k2 == NK2 - 1))
                        ot = gop.tile([128, 512], BF, tag='ot')
                        xres = xh[:, i, csl]
                        if has_g2:
                            xg2 = gop.tile([128, 512], FP32, tag='xg2')
                            nc.vector.tensor_tensor(out=xg2, in0=xh[:, i, csl],
                                                    in1=g2b[:, csl],
                                                    op=AluOpType.mult)
                            xres = xg2
                        if ch == 0:
                            nc.vector.scalar_tensor_tensor(
                                out=ot, in0=po, scalar=IWS, in1=xres,
                                op0=AluOpType.mult, op1=AluOpType.add)
                        else:
                            # off the vector engine: scale on scalar, add on pool
                            ot1 = gop.tile([128, 512], BF, tag='ot1')
                            nc.scalar.activation(out=ot1, in_=po, func=AF.Identity,
                                                 scale=IWS)
                            nc.gpsimd.tensor_tensor(out=ot, in0=ot1, in1=xres,
                                                    op=AluOpType.add)
                        if has_b2o:
                            nc.vector.tensor_tensor(out=ot, in0=ot,
                                                    in1=b2ob[:, csl],
                                                    op=AluOpType.add)
                        qeng = nc.scalar if ch else nc.sync
                        qeng.dma_start(out=d_out[isl, csl], in_=ot)

                emit_attn_ln2(0)
                emit_attn_ln2(1)
                emit_attn_ln2(2)
                emit_attn_ln2(3)
                emit_xh_transpose(slice(0, 512), 0, NJ)
                emit_attn_ln2(4)
                emit_o(0)
                emit_attn_ln2(5)
                emit_o(1)
                emit_attn_ln2(6)
                emit_o(2)
                emit_attn_ln2(7)
                emit_o(3)
                emit_xh_transpose(slice(512, 1024), 0, NJ)
                emit_o(4)
                emit_o(5)
                emit_o(6)
                emit_o(7)

    nc.compile()
    return nc


def _get_program(flags):
    if flags not in _BUILD_CACHE:
        _BUILD_CACHE[flags] = _build_program(flags)
    return _BUILD_CACHE[flags]


def _phm_weight(A, S):
    f = A.shape[0]
    din, dout = f * S.shape[1], f * S.shape[2]
    W = np.einsum('nij,nkl->ikjl', np.asarray(A, np.float32), np.asarray(S, np.float32))
    return np.ascontiguousarray(W.reshape(din, dout))


def _w8(W):
    """[E, E] fp32 -> [128, NK2, 2, E] fp8 with x64 scale."""
    Wv = (W * WS).reshape(NK2, 2, 128, E)
    return np.ascontiguousarray(np.transpose(Wv, (2, 0, 1, 3))).astype(E4M3)


def kernel(**inputs):
    query = np.asarray(inputs['query'], np.float32)
    g1 = np.asarray(inputs['g1'], np.float32)
    b1 = np.asarray(inputs['b1'], np.float32)
    g2 = np.asarray(inputs['g2'], np.float32)
    b2 = np.asarray(inputs['b2'], np.float32)
    qb = np.asarray(inputs['qb'], np.float32)
    kb = np.asarray(inputs['kb'], np.float32)
    vb = np.asarray(inputs['vb'], np.float32)
    ob = np.asarray(inputs['ob'], np.float32)

    Wq = _phm_weight(inputs['qA'], inputs['qS'])
    Wk = _phm_weight(inputs['kA'], inputs['kS'])
    Wv = _phm_weight(inputs['vA'], inputs['vS'])
    Wo = _phm_weight(inputs['oA'], inputs['oS'])

    has_g1b1 = not (np.all(g1 == 1.0) and np.all(b1 == 0.0))
    has_qb = bool(np.any(qb != 0.0))
    has_kb = bool(np.any(kb != 0.0))
    has_vb = bool(np.any(vb != 0.0))
    has_g2 = not np.all(g2 == 1.0)
    # final = xh@(g2*Wo) + xh*g2 + C,  C = b2@Wo + ob + b2
    C = b2 @ Wo + ob + b2
    has_b2o = bool(np.any(C != 0.0))
    general_q = has_g1b1 or has_qb
    flags = (has_g1b1, has_qb, has_kb, has_vb, has_g2, has_b2o)

    nc = _get_program(flags)

    Wg = g2[:, None] * Wo
    Wq_eff = g1[:, None] * Wq if has_g1b1 else Wq
    wq_b = _w8(Wq_eff)
    wk_b = _w8(Wk)
    wv_b = _w8(Wv)
    wo_b = _w8(Wg)

    s_full = np.sin((np.pi / 2) * np.arange(1, L + 1, dtype=np.float32) / L)
    c_full = np.cos((np.pi / 2) * np.arange(1, L + 1, dtype=np.float32) / L)

    in_maps = []
    for core in range(NCORES):
        b = core // 2
        l0 = (core % 2) * T
        x = np.ascontiguousarray(query[l0:l0 + T, b, :])
        xT = np.transpose(x).reshape(NK2, 2, 128, T)
        s = s_full[l0:l0 + T]
        c = c_full[l0:l0 + T]
        im = {
            'x_tm': x.astype(BF16),
            'x_fm8': np.ascontiguousarray(np.transpose(xT, (2, 0, 1, 3))).astype(E4M3),
            'wq8': wq_b, 'wk8': wk_b, 'wv8': wv_b, 'wo8': wo_b,
            's_bcast': np.ascontiguousarray(
                np.broadcast_to(s / WS, (128, T))).astype(BF16),
            'c_bcast': np.ascontiguousarray(
                np.broadcast_to(c / WS, (128, T))).astype(BF16),
            's_cols': np.ascontiguousarray((s / 8.0).reshape(NT, 128).T),
            'c_cols': np.ascontiguousarray((c / 8.0).reshape(NT, 128).T),
        }
        if has_g1b1:
            im['g1_b'] = np.ascontiguousarray(np.broadcast_to(g1, (128, E)))
            im['b1_b'] = np.ascontiguousarray(np.broadcast_to(b1, (128, E)))
        if general_q:
            cj = ((b1 @ Wq if has_g1b1 else np.zeros(E, np.float32)) + qb) * WS
            im['q_cj'] = np.ascontiguousarray(cj.reshape(NJ, 128).T)
        if has_kb:
            im['kb_b'] = np.ascontiguousarray(np.broadcast_to(kb * WS, (128, E)))
        if has_vb:
            im['vb_b'] = np.ascontiguousarray(np.broadcast_to(vb * WS, (128, E)))
        if has_g2:
            im['g2_b'] = np.ascontiguousarray(np.broadcast_to(g2, (128, E)))
        if has_b2o:
            im['b2o_b'] = np.ascontiguousarray(np.broadcast_to(C, (128, E)))
        in_maps.append(im)

    trace = bool(os.environ.get('KERNEL_TRACE'))
    res = run_bass_kernel_spmd(nc, in_maps, list(range(NCORES)), trace=trace)
    kernel._last_exec_ns = res.exec_time_ns

    out = np.empty((L, N, E), np.float32)
    for core in range(NCORES):
        b = core // 2
        l0 = (core % 2) * T
        out[l0:l0 + T, b, :] = res.results[core]['out'].astype(np.float32)
    return out


kernel._last_exec_ns = None


# revision 23
# speedup vs baseline: 1.7145x; 1.0257x over previous
"""Cosformer attention Bass kernel for 8 trn2 NeuronCores — fp8 edition.

Sharding: core c handles batch c//2, sequence half c%2 (1024 tokens).
Per-head linear-attention state (kv, ksum) is AllReduce'd (bf16) between
the two cores sharing a batch.

The 4 big projection matmuls run in fp8e4m3 with DoubleRow perf mode
(K=256 per instruction, ~2x bf16 rate). Weights are scaled x64 on host to
escape the fp8 subnormal zone; the scales cancel through the cosformer
z-normalization or are folded into epilogues.

LN1 folding: rstd cancels through z-normalization (relu commutes with
positive per-token scale), so the Q projection consumes centered x only:
x_c = x - mu, built feature-major from a partition-broadcast of mu. This
removes the qn transposes entirely; qn stays SBUF-resident for the
residual.

The final residual (+ out) is applied as an epilogue add (NOT folded into
Wo: the identity diagonal would quantize at ~6% in fp8).
"""

import sys

for _p in ('/opt/trn_rl_repo',):
    if _p not in sys.path:
        sys.path.insert(0, _p)

import importlib.util as _ilu
import os

os.environ.setdefault('NEURON_RT_RESET_CORES', '1')

# The image's antenv lacks axon_hooks (needed for trace=True); register ours.
if 'antenv.axon_hooks' not in sys.modules:
    _hp = '/opt/trn_rl_repo/antenv/axon_hooks.py'
    if os.path.exists(_hp):
        _spec = _ilu.spec_from_file_location('antenv.axon_hooks', _hp)
        _mod = _ilu.module_from_spec(_spec)
        _spec.loader.exec_module(_mod)
        sys.modules['antenv.axon_hooks'] = _mod
    else:
        import types as _types

        _mod = _types.ModuleType('antenv.axon_hooks')
        _mod._hook = None
        _mod.set_axon_ntff_profile_hook = lambda h: setattr(_mod, '_hook', h)
        _mod.get_axon_ntff_profile_hook = lambda: _mod._hook
        sys.modules['antenv.axon_hooks'] = _mod


def _register_ntff_hook():
    """If boot didn't register the NTFF profile hook (image antenv lacks
    axon_hooks), drive NRT profiling via ctypes into libaxon_pjrt.so."""
    import contextlib
    import ctypes

    mod = sys.modules['antenv.axon_hooks']
    if mod.get_axon_ntff_profile_hook() is not None:
        return
    so_path = '/opt/axon/libaxon_pjrt.so'
    if not os.path.exists(so_path):
        return
    try:
        lib = ctypes.CDLL(so_path)
        if not hasattr(lib, 'axon_start_nrt_profile'):
            return
        lib.axon_start_nrt_profile.argtypes = [
            ctypes.POINTER(ctypes.c_int64), ctypes.c_size_t]
        lib.axon_start_nrt_profile.restype = ctypes.c_int64
        lib.axon_stop_nrt_profile.argtypes = [ctypes.c_char_p]
        lib.axon_stop_nrt_profile.restype = ctypes.c_int64
    except OSError:
        return

    @contextlib.contextmanager
    def _hook(output_dir, device_ids):
        import jax
        jax.devices()
        if device_ids:
            ids = (ctypes.c_int64 * len(device_ids))(*device_ids)
            rc = lib.axon_start_nrt_profile(ids, len(device_ids))
        else:
            rc = lib.axon_start_nrt_profile(None, 0)
        if rc != 0:
            raise RuntimeError(f'axon_start_nrt_profile rc={rc}')
        try:
            yield
        finally:
            n = lib.axon_stop_nrt_profile(str(output_dir).encode())
            if n < 0:
                raise RuntimeError(f'axon_stop_nrt_profile rc={n}')

    mod.set_axon_ntff_profile_hook(_hook)


_register_ntff_hook()

import numpy as np
import ml_dtypes

import concourse.bass as bass
import concourse.tile as tile
from concourse import bacc, mybir
from concourse.alu_op_type import AluOpType
from concourse.bass_utils import run_bass_kernel_spmd

BF16 = ml_dtypes.bfloat16
E4M3 = ml_dtypes.float8_e4m3
FP32 = mybir.dt.float32
BF = mybir.dt.bfloat16
F8 = mybir.dt.float8e4
AF = mybir.ActivationFunctionType
DR = mybir.MatmulPerfMode.DoubleRow

L, N, E, H, D = 2048, 4, 1024, 16, 64
T = 1024            # tokens per core
NT = T // 128       # 8 token tiles
NK2 = 4             # DoubleRow contraction steps (256 features each)
NJ = E // 128       # 8 output-feature tiles
NCORES = 8
EPS_LN = 1e-5
EPS_ATTN = 1e-6
WS = 64.0           # host weight scale
IWS = 1.0 / WS

_BUILD_CACHE = {}


def _build_program(flags):
    """flags: (has_g1b1, has_qb, has_kb, has_vb, has_g2, has_b2o)."""
    has_g1b1, has_qb, has_kb, has_vb, has_g2, has_b2o = flags
    general_q = has_g1b1 or has_qb   # r no longer cancels for the Q path

    nc = bacc.Bacc("TRN2", target_bir_lowering=False, debug=False,
                   num_devices=NCORES)

    # ---- DRAM I/O ----
    d_x_tm = nc.dram_tensor('x_tm', [T, E], BF, kind='ExternalInput')
    d_x_fm8 = nc.dram_tensor('x_fm8', [128, NK2, 2, T], F8, kind='ExternalInput')
    d_wq = nc.dram_tensor('wq8', [128, NK2, 2, E], F8, kind='ExternalInput')
    d_wk = nc.dram_tensor('wk8', [128, NK2, 2, E], F8, kind='ExternalInput')
    d_wv = nc.dram_tensor('wv8', [128, NK2, 2, E], F8, kind='ExternalInput')
    d_wo = nc.dram_tensor('wo8', [128, NK2, 2, E], F8, kind='ExternalInput')
    d_sb = nc.dram_tensor('s_bcast', [128, T], BF, kind='ExternalInput')
    d_cb = nc.dram_tensor('c_bcast', [128, T], BF, kind='ExternalInput')
    d_scol = nc.dram_tensor('s_cols', [128, NT], FP32, kind='ExternalInput')
    d_ccol = nc.dram_tensor('c_cols', [128, NT], FP32, kind='ExternalInput')
    d_g1b = nc.dram_tensor('g1_b', [128, E], FP32, kind='ExternalInput') if has_g1b1 else None
    d_b1b = nc.dram_tensor('b1_b', [128, E], FP32, kind='ExternalInput') if has_g1b1 else None
    d_qcj = nc.dram_tensor('q_cj', [128, NJ], FP32, kind='ExternalInput') if general_q else None
    d_kbb = nc.dram_tensor('kb_b', [128, E], FP32, kind='ExternalInput') if has_kb else None
    d_vbb = nc.dram_tensor('vb_b', [128, E], FP32, kind='ExternalInput') if has_vb else None
    d_g2b = nc.dram_tensor('g2_b', [128, E], FP32, kind='ExternalInput') if has_g2 else None
    d_b2ob = nc.dram_tensor('b2o_b', [128, E], FP32, kind='ExternalInput') if has_b2o else None
    d_out = nc.dram_tensor('out', [T, E], BF, kind='ExternalOutput')

    RG = [[0, 1], [2, 3], [4, 5], [6, 7]]

    with tile.TileContext(nc) as tc:
        with (
            tc.tile_pool(name='persist', bufs=1) as pp,
            tc.tile_pool(name='wpool', bufs=2) as wp,
            tc.tile_pool(name='dram', bufs=1, space='DRAM') as dp,
        ):
            # ---- persistent tiles ----
            sbt = pp.tile([128, T], BF, tag='sbt')          # s/WS bcast
            cbt = pp.tile([128, T], BF, tag='cbt')
            scol = pp.tile([128, NT], FP32, tag='scol')     # s/8 cols
            ccol = pp.tile([128, NT], FP32, tag='ccol')
            eps1 = pp.tile([128, 1], FP32, tag='eps1')
            qn = pp.tile([128, NT, E], BF, tag='qn')        # LN1 out, token-major
            xh = pp.tile([128, NT, E], BF, tag='xh')        # LN2 out, token-major
            xc8 = pp.tile([128, NK2, 2, T], F8, tag='xc8')  # centered x, fm
            ksc = pp.tile([128, NT, H, 128], F8, tag='ksc') # 8*k*[s|c], tok-major
            v_aug = pp.tile([128, NT, H, 65], F8, tag='vaug')
            qq = pp.tile([128, H, T], BF, tag='qq')         # q~*[s|c] per head, fm
            kvb = pp.tile([128, H * 65], BF, tag='kvb')     # reduced kv (true scale)
            kvp = pp.tile([128, H * 65], BF, tag='kvp')
            xhT = pp.tile([128, NJ, T], BF, tag='xhT')      # transposed xh
            xhT8 = pp.tile([128, NK2, 2, T], F8, tag='xhT8')
            negmu_cols = pp.tile([128, NT], BF, tag='nmu')  # -mu per token
            mu_row = pp.tile([1, T], BF, tag='murow')
            mu_bc = pp.tile([128, T], BF, tag='mubc')
            rst_cols = pp.tile([128, NT], FP32, tag='rst') if general_q else None
            r_row = pp.tile([1, T], FP32, tag='rrow') if general_q else None
            r_bc = pp.tile([128, T], FP32, tag='rbc') if general_q else None

            g1b = b1b = qcj = kbb = vbb = g2b = b2ob = None
            if has_g1b1:
                g1b = pp.tile([128, E], FP32, tag='g1b')
                b1b = pp.tile([128, E], FP32, tag='b1b')
                nc.gpsimd.dma_start(out=g1b, in_=d_g1b[:])
                nc.gpsimd.dma_start(out=b1b, in_=d_b1b[:])
            if general_q:
                qcj = pp.tile([128, NJ], FP32, tag='qcj')
                nc.gpsimd.dma_start(out=qcj, in_=d_qcj[:])
            if has_kb:
                kbb = pp.tile([128, E], FP32, tag='kbb')
                nc.gpsimd.dma_start(out=kbb, in_=d_kbb[:])
            if has_vb:
                vbb = pp.tile([128, E], FP32, tag='vbb')
                nc.gpsimd.dma_start(out=vbb, in_=d_vbb[:])
            if has_g2:
                g2b = pp.tile([128, E], FP32, tag='g2b')
                nc.gpsimd.dma_start(out=g2b, in_=d_g2b[:])
            if has_b2o:
                b2ob = pp.tile([128, E], FP32, tag='b2ob')
                nc.gpsimd.dma_start(out=b2ob, in_=d_b2ob[:])

            # DRAM scratch
            xh_dram = dp.tile([T, E], BF)
            kv_cc_in = dp.tile([128, H * 65], BF)
            kv_cc_out = dp.tile([128, H * 65], BF)

            # ---- front-loaded DMAs ----
            # scalar queue: x_tm tiles (LN1 needs them first) interleaved
            # with Wk chunks (PE needs chunk 0 at ~1.5us)
            xfm = pp.tile([128, NK2, 2, T], F8, tag='xfm')
            wk_t = wp.tile([128, NK2, 2, E], F8, tag='W')
            with tc.tile_pool(name='xtiles', bufs=8) as xp:
                xts = []
                for i in range(NT):
                    xt = xp.tile([128, E], BF, tag='xt', name=f'xt{i}')
                    xts.append(xt)
                nc.scalar.dma_start(out=xts[0], in_=d_x_tm[0:128, :])
                nc.scalar.dma_start(out=wk_t[:, 0], in_=d_wk[:, 0])
                nc.scalar.dma_start(out=xts[1], in_=d_x_tm[128:256, :])
                nc.scalar.dma_start(out=wk_t[:, 1], in_=d_wk[:, 1])
                for k2 in range(NK2):
                    nc.sync.dma_start(out=xfm[:, k2], in_=d_x_fm8[:, k2])
                nc.scalar.dma_start(out=wk_t[:, 2], in_=d_wk[:, 2])
                nc.scalar.dma_start(out=wk_t[:, 3], in_=d_wk[:, 3])
                for i in range(2, NT):
                    nc.scalar.dma_start(out=xts[i],
                                        in_=d_x_tm[i * 128:(i + 1) * 128, :])
                nc.sync.dma_start(out=scol, in_=d_scol[:])
                nc.sync.dma_start(out=ccol, in_=d_ccol[:])
                nc.sync.dma_start(out=sbt, in_=d_sb[:])
                nc.sync.dma_start(out=cbt, in_=d_cb[:])
                nc.vector.memset(eps1, EPS_LN)
                nc.vector.memset(v_aug[:, :, :, 64:65], 8.0)
                wv_t = wp.tile([128, NK2, 2, E], F8, tag='W')
                for k2 in range(NK2):
                    nc.gpsimd.dma_start(out=wv_t[:, k2], in_=d_wv[:, k2])

                # ---- Phase A: LN1 per token tile (vector stats, scalar app) ----
                with tc.tile_pool(name='ln1', bufs=4) as ap:
                    for i in range(NT):
                        xt = xts[i]
                        st = ap.tile([128, 2, 6], FP32, tag='st')
                        xg = xt[:].rearrange('p (g d) -> p g d', g=2)
                        nc.vector.bn_stats(out=st[:, 0, :], in_=xg[:, 0, :])
                        nc.vector.bn_stats(out=st[:, 1, :], in_=xg[:, 1, :])
                        mv = ap.tile([128, 2], FP32, tag='mv')
                        nc.vector.bn_aggr(out=mv, in_=st)
                        rstd = ap.tile([128, 1], FP32, tag='rstd')
                        nc.scalar.activation(out=rstd, in_=mv[:, 1:2], func=AF.Sqrt,
                                             bias=eps1, scale=1.0)
                        nc.vector.reciprocal(out=rstd, in_=rstd)
                        # -mu (bf16) for the x_c broadcast path
                        nc.vector.tensor_scalar(out=negmu_cols[:, i:i + 1],
                                                in0=mv[:, 0:1], scalar1=-1.0,
                                                scalar2=None, op0=AluOpType.mult)
                        if general_q:
                            nc.vector.tensor_copy(out=rst_cols[:, i:i + 1],
                                                  in_=rstd)
                        # qn = (x - mu) * rstd (+affine) via scalar ACT
                        nbias = ap.tile([128, 1], FP32, tag='nb')
                        nc.vector.tensor_scalar(out=nbias, in0=mv[:, 0:1],
                                                scalar1=rstd, scalar2=-1.0,
                                                op0=AluOpType.mult,
                                                op1=AluOpType.mult)
                        if has_g1b1:
                            qtmp = ap.tile([128, E], FP32, tag='qtmp')
                            nc.scalar.activation(out=qtmp, in_=xt,
                                                 func=AF.Identity,
                                                 bias=nbias, scale=rstd)
                            nc.vector.scalar_tensor_tensor(
                                out=qn[:, i, :], in0=qtmp, scalar=1.0, in1=g1b,
                                op0=AluOpType.mult, op1=AluOpType.mult)
                            nc.vector.tensor_tensor(out=qn[:, i, :],
                                                    in0=qn[:, i, :], in1=b1b,
                                                    op=AluOpType.add)
                        else:
                            nc.scalar.activation(out=qn[:, i, :], in_=xt,
                                                 func=AF.Identity, bias=nbias,
                                                 scale=rstd)
                        # gather -mu into a flat [1, T] row (cross-partition DMA)
                        nc.sync.dma_start(
                            out=mu_row[:, i * 128:(i + 1) * 128],
                            in_=negmu_cols[:, i:i + 1])
                        if general_q:
                            nc.sync.dma_start(
                                out=r_row[:, i * 128:(i + 1) * 128],
                                in_=rst_cols[:, i:i + 1])

                # mu flat row -> partition broadcast -> centered x (fp8)
                nc.gpsimd.partition_broadcast(mu_bc, mu_row, channels=128)
                for k2 in range(NK2):
                    for h in range(2):
                        nc.vector.tensor_tensor(out=xc8[:, k2, h, :],
                                                in0=xfm[:, k2, h, :],
                                                in1=mu_bc, op=AluOpType.add)
                if general_q:
                    nc.gpsimd.partition_broadcast(r_bc, r_row, channels=128)

                # ---- Phases B1/B2: K and V projections (fp8 DR) ----
                with tc.tile_pool(name='psB', bufs=8, space='PSUM') as psb:
                    def phm_tok_major(w_t, epilogue, nm):
                        for half in range(2):
                            ptiles = {}
                            for i in range(4 * half, 4 * half + 4):
                                for ch in range(2):
                                    pt = psb.tile([128, 512], FP32, tag='psB',
                                                  name=f'pb{nm}_{i}_{ch}')
                                    ptiles[i, ch] = pt
                            for k2 in range(NK2):
                                for i in range(4 * half, 4 * half + 4):
                                    isl = slice(i * 128, (i + 1) * 128)
                                    for ch in range(2):
                                        csl = slice(ch * 512, (ch + 1) * 512)
                                        nc.tensor.matmul(
                                            ptiles[i, ch],
                                            lhsT=xfm[:, k2, :, isl],
                                            rhs=w_t[:, k2, :, csl],
                                            perf_mode=DR,
                                            start=(k2 == 0), stop=(k2 == NK2 - 1))
                            for i in range(4 * half, 4 * half + 4):
                                for ch in range(2):
                                    epilogue(i, ch, ptiles[i, ch])

                    def k_epilogue(i, ch, pk):
                        csl = slice(ch * 512, (ch + 1) * 512)
                        if has_kb:
                            nc.vector.tensor_tensor(out=pk, in0=pk,
                                                    in1=kbb[:, csl],
                                                    op=AluOpType.add)
                        pkv = pk[:].rearrange('p (h d) -> p h d', d=64)
                        hsl = slice(ch * 8, (ch + 1) * 8)
                        # sin half on scalar engine, cos half on vector
                        nc.scalar.activation(
                            out=ksc[:, i, hsl, 0:64],
                            in_=pk[:].rearrange('p (h d) -> p h d', d=64),
                            func=AF.Relu, scale=scol[:, i:i + 1])
                        nc.vector.tensor_scalar(
                            out=ksc[:, i, hsl, 64:128], in0=pkv,
                            scalar1=0.0, scalar2=ccol[:, i:i + 1],
                            op0=AluOpType.max, op1=AluOpType.mult)

                    def v_epilogue(i, ch, pv):
                        csl = slice(ch * 512, (ch + 1) * 512)
                        if has_vb:
                            nc.vector.tensor_tensor(out=pv, in0=pv,
                                                    in1=vbb[:, csl],
                                                    op=AluOpType.add)
                        hsl = slice(ch * 8, (ch + 1) * 8)
                        nc.vector.tensor_scalar(
                            out=v_aug[:, i, hsl, 0:64],
                            in0=pv[:].rearrange('p (h d) -> p h d', d=64),
                            scalar1=0.125, scalar2=None, op0=AluOpType.mult)

                    phm_tok_major(wk_t, k_epilogue, 'k')
                    phm_tok_major(wv_t, v_epilogue, 'v')

                # Wq load (wk slot frees): gpsimd queue (scalar is busy with
                # epilogue ACTs; gpsimd is idle here)
                wq_t = wp.tile([128, NK2, 2, E], F8, tag='W')
                for k2 in range(NK2):
                    nc.gpsimd.dma_start(out=wq_t[:, k2], in_=d_wq[:, k2])

                # ---- Phase C: per-head kv partials (fp8 DR) + AllReduce ----
                with tc.tile_pool(name='psC', bufs=8, space='PSUM') as psc:
                    for h in range(H):
                        pkv = psc.tile([128, 65], FP32, tag='psC', name=f'kv{h}')
                        for i2 in range(NT // 2):
                            nc.tensor.matmul(
                                pkv,
                                lhsT=ksc[:, 2 * i2:2 * i2 + 2, h, :],
                                rhs=v_aug[:, 2 * i2:2 * i2 + 2, h, :],
                                perf_mode=DR,
                                start=(i2 == 0), stop=(i2 == NT // 2 - 1))
                        nc.vector.tensor_scalar(
                            out=kvp[:, h * 65:(h + 1) * 65], in0=pkv,
                            scalar1=IWS, scalar2=None, op0=AluOpType.mult)

                nc.gpsimd.dma_start(out=kv_cc_in[:], in_=kvp)
                nc.gpsimd.collective_compute(
                    'AllReduce', AluOpType.add,
                    ins=[kv_cc_in.opt()], outs=[kv_cc_out.opt()],
                    replica_groups=RG)

            # Wo load (wv slot frees): gpsimd queue, during the collective
            wo_t = wp.tile([128, NK2, 2, E], F8, tag='W')
            for k2 in range(NK2):
                nc.gpsimd.dma_start(out=wo_t[:, k2], in_=d_wo[:, k2])
            nc.gpsimd.dma_start(out=kvb, in_=kv_cc_out[:])

            # ---- Phase D: Q projection (fp8 DR from x_c) ----
            with (
                tc.tile_pool(name='psD', bufs=4, space='PSUM') as psd,
                tc.tile_pool(name='qsb', bufs=4) as qsb,
            ):
                for ch in range(2):
                    csl = slice(ch * 512, (ch + 1) * 512)
                    for j in range(NJ):
                        jsl = slice(j * 128, (j + 1) * 128)
                        pq = psd.tile([128, 512], FP32, tag='psD',
                                      name=f'pq{ch}_{j}')
                        for k2 in range(NK2):
                            nc.tensor.matmul(
                                pq, lhsT=wq_t[:, k2, :, jsl],
                                rhs=xc8[:, k2, :, csl],
                                perf_mode=DR,
                                start=(k2 == 0), stop=(k2 == NK2 - 1))
                        if general_q:
                            nc.vector.tensor_tensor(out=pq, in0=pq,
                                                    in1=r_bc[:, csl],
                                                    op=AluOpType.mult)
                        qrel = qsb.tile([128, 512], BF, tag='qrel')
                        if general_q:
                            nc.scalar.activation(out=qrel, in_=pq, func=AF.Relu,
                                                 bias=qcj[:, j:j + 1])
                        else:
                            nc.scalar.activation(out=qrel, in_=pq, func=AF.Relu)
                        for hh in range(2):
                            h = 2 * j + hh
                            rs = slice(hh * 64, (hh + 1) * 64)
                            nc.vector.tensor_tensor(
                                out=qq[0:64, h, csl], in0=qrel[rs, :],
                                in1=sbt[rs, csl], op=AluOpType.mult)
                            nc.vector.tensor_tensor(
                                out=qq[64:128, h, csl], in0=qrel[rs, :],
                                in1=cbt[rs, csl], op=AluOpType.mult)

            # ---- Phases E/F/G interleaved ----
            with (
                tc.tile_pool(name='ef', bufs=3) as efp,
                tc.tile_pool(name='psE', bufs=4, space='PSUM') as pse,
                tc.tile_pool(name='go', bufs=4) as gop,
                tc.tile_pool(name='psG', bufs=4, space='PSUM') as psg,
            ):
                def emit_attn_ln2(i):
                    rsl = slice(i * 128, (i + 1) * 128)
                    yt = efp.tile([128, H, 64], BF, tag='yt')
                    z16 = efp.tile([128, H], FP32, tag='z16')
                    pas = []
                    for g in range(4):
                        pa = pse.tile([128, 4 * 65], FP32, tag='psE',
                                      name=f'pa_{i}_{g}')
                        pas.append(pa)
                        for hh in range(4):
                            h = 4 * g + hh
                            nc.tensor.matmul(pa[:, hh * 65:(hh + 1) * 65],
                                             lhsT=qq[:, h, rsl],
                                             rhs=kvb[:, h * 65:(h + 1) * 65],
                                             start=True, stop=True)
                        pav = pa[:].rearrange('p (h c) -> p h c', c=65)
                        nc.vector.tensor_scalar(
                            out=z16[:, g * 4:(g + 1) * 4], in0=pav[:, :, 64],
                            scalar1=EPS_ATTN, scalar2=None, op0=AluOpType.max)
                    nc.vector.reciprocal(out=z16, in_=z16)
                    for g in range(4):
                        pav = pas[g][:].rearrange('p (h c) -> p h c', c=65)
                        zb = z16[:, g * 4:(g + 1) * 4].broadcast_to((128, 4, 64))
                        nc.vector.tensor_tensor(out=yt[:, g * 4:(g + 1) * 4, :],
                                                in0=pav[:, :, 0:64], in1=zb,
                                                op=AluOpType.mult)
                    ytf = yt[:].rearrange('p h d -> p (h d)')
                    nc.vector.tensor_tensor(out=ytf, in0=ytf, in1=qn[:, i, :],
                                            op=AluOpType.add)
                    # LN2 stats
                    st2 = efp.tile([128, 2, 6], FP32, tag='st2')
                    yg = yt[:].rearrange('p (g x) d -> p g (x d)', g=2)
                    nc.vector.bn_stats(out=st2[:, 0, :], in_=yg[:, 0, :])
                    nc.vector.bn_stats(out=st2[:, 1, :], in_=yg[:, 1, :])
                    mv2 = efp.tile([128, 2], FP32, tag='mv2')
                    nc.vector.bn_aggr(out=mv2, in_=st2)
                    mu2c = mv2[:, 0:1]
                    rstd2 = efp.tile([128, 1], FP32, tag='rstd2')
                    nc.scalar.activation(out=rstd2, in_=mv2[:, 1:2], func=AF.Sqrt,
                                         bias=eps1, scale=1.0)
                    nc.vector.reciprocal(out=rstd2, in_=rstd2)
                    nb2 = efp.tile([128, 1], FP32, tag='nb2')
                    nc.vector.tensor_scalar(out=nb2, in0=mu2c,
                                            scalar1=rstd2, scalar2=-1.0,
                                            op0=AluOpType.mult,
                                            op1=AluOpType.mult)
                    nc.scalar.activation(out=xh[:, i, :], in_=ytf, func=AF.Identity,
                                         bias=nb2, scale=rstd2)
                    nc.gpsimd.dma_start(out=xh_dram[rsl, :], in_=xh[:, i, :])

                def emit_xh_transpose(tsl, jlo, jhi):
                    for j in range(jlo, jhi):
                        qeng = nc.sync if j % 2 else nc.scalar
                        qeng.dma_start(out=xhT[:, j, tsl],
                                       in_=xh_dram[tsl, j * 128:(j + 1) * 128],
                                       transpose=True)
                    for j in range(jlo, jhi):
                        nc.vector.tensor_copy(
                            out=xhT8[:, j // 2, j % 2, tsl],
                            in_=xhT[:, j, tsl])

                def emit_o(i):
                    isl = slice(i * 128, (i + 1) * 128)
                    for ch in range(2):
                        csl = slice(ch * 512, (ch + 1) * 512)
                        po = psg.tile([128, 512], FP32, tag='psG',
                                      name=f'po_{i}_{ch}')
                        for k2 in range(NK2):
                            nc.tensor.matmul(
                                po, lhsT=xhT8[:, k2, :, isl],
                                rhs=wo_t[:, k2, :, csl],
                                perf_mode=DR,
                                start=(k2 == 0), stop=(k2 == NK2 - 1))
                        ot = gop.tile([128, 512], BF, tag='ot')
                        xres = xh[:, i, csl]
                        if has_g2:
                            xg2 = gop.tile([128, 512], FP32, tag='xg2')
                            nc.vector.tensor_tensor(out=xg2, in0=xh[:, i, csl],
                                                    in1=g2b[:, csl],
                                                    op=AluOpType.mult)
                            xres = xg2
                        nc.vector.scalar_tensor_tensor(
                            out=ot, in0=po, scalar=IWS, in1=xres,
                            op0=AluOpType.mult, op1=AluOpType.add)
                        if has_b2o:
                            nc.vector.tensor_tensor(out=ot, in0=ot,
                                                    in1=b2ob[:, csl],
                                                    op=AluOpType.add)
                        qeng = nc.scalar if ch else nc.sync
                        qeng.dma_start(out=d_out[isl, csl], in_=ot)

                emit_attn_ln2(0)
                emit_attn_ln2(1)
                emit_attn_ln2(2)
                emit_attn_ln2(3)
                emit_xh_transpose(slice(0, 512), 0, NJ)
                emit_attn_ln2(4)
                emit_o(0)
                emit_attn_ln2(5)
                emit_o(1)
                emit_attn_ln2(6)
                emit_o(2)
                emit_attn_ln2(7)
                emit_o(3)
                emit_xh_transpose(slice(512, 1024), 0, NJ)
                emit_o(4)
                emit_o(5)
                emit_o(6)
                emit_o(7)

    nc.compile()
    return nc


def _get_program(flags):
    if flags not in _BUILD_CACHE:
        _BUILD_CACHE[flags] = _build_program(flags)
    return _BUILD_CACHE[flags]


def _phm_weight(A, S):
    f = A.shape[0]
    din, dout = f * S.shape[1], f * S.shape[2]
    W = np.einsum('nij,nkl->ikjl', np.asarray(A, np.float32), np.asarray(S, np.float32))
    return np.ascontiguousarray(W.reshape(din, dout))


def _w8(W):
    """[E, E] fp32 -> [128, NK2, 2, E] fp8 with x64 scale."""
    Wv = (W * WS).reshape(NK2, 2, 128, E)
    return np.ascontiguousarray(np.transpose(Wv, (2, 0, 1, 3))).astype(E4M3)


def kernel(**inputs):
    query = np.asarray(inputs['query'], np.float32)
    g1 = np.asarray(inputs['g1'], np.float32)
    b1 = np.asarray(inputs['b1'], np.float32)
    g2 = np.asarray(inputs['g2'], np.float32)
    b2 = np.asarray(inputs['b2'], np.float32)
    qb = np.asarray(inputs['qb'], np.float32)
    kb = np.asarray(inputs['kb'], np.float32)
    vb = np.asarray(inputs['vb'], np.float32)
    ob = np.asarray(inputs['ob'], np.float32)

    Wq = _phm_weight(inputs['qA'], inputs['qS'])
    Wk = _phm_weight(inputs['kA'], inputs['kS'])
    Wv = _phm_weight(inputs['vA'], inputs['vS'])
    Wo = _phm_weight(inputs['oA'], inputs['oS'])

    has_g1b1 = not (np.all(g1 == 1.0) and np.all(b1 == 0.0))
    has_qb = bool(np.any(qb != 0.0))
    has_kb = bool(np.any(kb != 0.0))
    has_vb = bool(np.any(vb != 0.0))
    has_g2 = not np.all(g2 == 1.0)
    # final = xh@(g2*Wo) + xh*g2 + C,  C = b2@Wo + ob + b2
    C = b2 @ Wo + ob + b2
    has_b2o = bool(np.any(C != 0.0))
    general_q = has_g1b1 or has_qb
    flags = (has_g1b1, has_qb, has_kb, has_vb, has_g2, has_b2o)

    nc = _get_program(flags)

    Wg = g2[:, None] * Wo
    Wq_eff = g1[:, None] * Wq if has_g1b1 else Wq
    wq_b = _w8(Wq_eff)
    wk_b = _w8(Wk)
    wv_b = _w8(Wv)
    wo_b = _w8(Wg)

    s_full = np.sin((np.pi / 2) * np.arange(1, L + 1, dtype=np.float32) / L)
    c_full = np.cos((np.pi / 2) * np.arange(1, L + 1, dtype=np.float32) / L)

    in_maps = []
    for core in range(NCORES):
        b = core // 2
        l0 = (core % 2) * T
        x = np.ascontiguousarray(query[l0:l0 + T, b, :])
        xT = np.transpose(x).reshape(NK2, 2, 128, T)
        s = s_full[l0:l0 + T]
        c = c_full[l0:l0 + T]
        im = {
            'x_tm': x.astype(BF16),
            'x_fm8': np.ascontiguousarray(np.transpose(xT, (2, 0, 1, 3))).astype(E4M3),
            'wq8': wq_b, 'wk8': wk_b, 'wv8': wv_b, 'wo8': wo_b,
            's_bcast': np.ascontiguousarray(
                np.broadcast_to(s / WS, (128, T))).astype(BF16),
            'c_bcast': np.ascontiguousarray(
                np.broadcast_to(c / WS, (128, T))).astype(BF16),
            's_cols': np.ascontiguousarray((s / 8.0).reshape(NT, 128).T),
            'c_cols': np.ascontiguousarray((c / 8.0).reshape(NT, 128).T),
        }
        if has_g1b1:
            im['g1_b'] = np.ascontiguousarray(np.broadcast_to(g1, (128, E)))
            im['b1_b'] = np.ascontiguousarray(np.broadcast_to(b1, (128, E)))
        if general_q:
            cj = ((b1 @ Wq if has_g1b1 else np.zeros(E, np.float32)) + qb) * WS
            im['q_cj'] = np.ascontiguousarray(cj.reshape(NJ, 128).T)
        if has_kb:
            im['kb_b'] = np.ascontiguousarray(np.broadcast_to(kb * WS, (128, E)))
        if has_vb:
            im['vb_b'] = np.ascontiguousarray(np.broadcast_to(vb * WS, (128, E)))
        if has_g2:
            im['g2_b'] = np.ascontiguousarray(np.broadcast_to(g2, (128, E)))
        if has_b2o:
            im['b2o_b'] = np.ascontiguousarray(np.broadcast_to(C, (128, E)))
        in_maps.append(im)

    trace = bool(os.environ.get('KERNEL_TRACE'))
    res = run_bass_kernel_spmd(nc, in_maps, list(range(NCORES)), trace=trace)
    kernel._last_exec_ns = res.exec_time_ns

    out = np.empty((L, N, E), np.float32)
    for core in range(NCORES):
        b = core // 2
        l0 = (core % 2) * T
        out[l0:l0 + T, b, :] = res.results[core]['out'].astype(np.float32)
    return out


kernel._last_exec_ns = None
